# revision 1
# baseline (speedup 1.0000x reference)
"""Trainium2 Bass kernel for nn_Decoder (GNN message passing):
LSTM(1 step) -> GCNConv -> ReLU -> GCNConv -> Linear -> ReLU on a
100K-node / 1.6M-edge graph, SPMD across 8 NeuronCores.

Strategy (dst-node sharding):
- Core c owns nodes [c*12500, (c+1)*12500) and all edges into them.
- Per-node compute (LSTM, x@W transforms) runs feature-major [128, nodes]
  so all matmuls need zero transposes and biases are per-partition.
- The GCN propagate gathers transformed rows from a bf16 node-major table
  in DRAM (built via AllGather of the 8 shards) with gpsimd.dma_gather,
  then scatter-adds via PE matmul with an on-chip selection matrix
  (tensor_scalar: iota==dst_idx -> * norm), accumulated in PSUM per
  128-dst block.
"""

from contextlib import ExitStack

import numpy as np
import ml_dtypes

import concourse.bacc as bacc
import concourse.mybir as mybir
import concourse.tile as tile
from concourse.bass_utils import run_bass_kernel_spmd

P = 128
N = 100000
NCORES = 8
NPC = N // NCORES            # 12500 nodes per core
NBLK = (NPC + P - 1) // P    # 98 dst blocks per core (last has 84)
CH = 4                       # src chunks (int16 gather index limit)
QROWS = NPC // CH            # 3125: per-rank quarter contributed to a chunk
CHROWS = QROWS * NCORES      # 25000 rows per chunk table
GT = 48                      # tiles (of 128 edges) per dma_gather
LSTM_CHUNK = 500             # nodes per LSTM/matmul column chunk

bf16 = ml_dtypes.bfloat16
f32 = np.float32


# ---------------------------------------------------------------- host prep


def _prep_edges(edge_index):
    """Sort/pad each core's incident edges into a cross-core-uniform tile
    schedule. Returns per-core device arrays + the static schedule."""
    src = np.asarray(edge_index[0], dtype=np.int64)
    dst = np.asarray(edge_index[1], dtype=np.int64)
    loops = np.arange(N, dtype=np.int64)
    src = np.concatenate([src, loops])
    dst = np.concatenate([dst, loops])

    deg = np.bincount(dst, minlength=N).astype(np.float64)
    dinv = 1.0 / np.sqrt(deg)
    norm = (dinv[src] * dinv[dst]).astype(np.float32)

    core_of = dst // NPC
    per_core = []
    counts = np.zeros((NCORES, CH, NBLK), np.int64)
    for c in range(NCORES):
        m = core_of == c
        s = src[m]
        d = dst[m] - c * NPC
        w = norm[m]
        ch = (s % NPC) // QROWS
        o = np.lexsort((d, ch))
        s, d, w, ch = s[o], d[o], w[o], ch[o]
        b = d // P
        counts[c] = np.bincount(ch * NBLK + b, minlength=CH * NBLK).reshape(
            CH, NBLK
        )
        per_core.append((s, d, w, ch, b))

    # tiles per (chunk, block) run: padded to the max across cores
    T_run = (counts.max(axis=0) + P - 1) // P          # [CH, NBLK]
    flat = T_run.reshape(-1)
    base = np.zeros(CH * NBLK + 1, np.int64)
    np.cumsum(flat, out=base[1:])                      # tile offset per run
    TT = int(base[-1])
    NIDX = TT * P
    ctb = [int(base[ch * NBLK]) for ch in range(CH)] + [TT]  # chunk tile base

    arrs = []
    for c in range(NCORES):
        s, d, w, ch, b = per_core[c]
        gid = ch * NBLK + b
        cnt = counts[c].reshape(-1)
        gstart = np.concatenate([[0], np.cumsum(cnt)[:-1]])
        within = np.arange(len(s)) - gstart[gid]
        pos = base[gid] * P + within

        idxs = np.zeros(NIDX, np.int16)                 # pad -> row 0 (valid)
        # chunk q table = concat over ranks of each rank's q-th quarter
        idxs[pos] = ((s // NPC) * QROWS + (s % QROWS)).astype(np.int16)
        dstv = np.full(NIDX, -1.0, np.float32)          # pad -> no dst match
        dstv[pos] = (d - b * P).astype(np.float32)
        nrmv = np.zeros(NIDX, np.float32)
        nrmv[pos] = w

        idx16 = np.tile(np.ascontiguousarray(idxs.reshape(-1, 16).T), (8, 1))
        dstt = np.ascontiguousarray(dstv.reshape(TT, P).T)
        nrmt = np.ascontiguousarray(nrmv.reshape(TT, P).T)
        arrs.append((idx16, dstt, nrmt))

    # gather pieces: per chunk, consecutive groups of <= GT tiles
    pieces = []
    for chn in range(CH):
        t0, t1 = ctb[chn], ctb[chn + 1]
        pieces.append([(t, min(GT, t1 - t)) for t in range(t0, t1, GT)])

    sched = dict(T_run=T_run, base=base, TT=TT, NIDX=NIDX, ctb=ctb, pieces=pieces)
    return arrs, sched


# ---------------------------------------------------------------- device


def _build_nc(sched):
    T_run, base, TT, NIDX, ctb, pieces = (
        sched["T_run"],
        sched["base"],
        sched["TT"],
        sched["NIDX"],
        sched["ctb"],
        sched["pieces"],
    )
    dt = mybir.dt
    alu = mybir.AluOpType
    act = mybir.ActivationFunctionType

    nc = bacc.Bacc("TRN2", target_bir_lowering=False, debug=False, num_devices=NCORES)

    # ---- I/O
    zT_d = nc.dram_tensor("zT", [P, NPC], dt.bfloat16, kind="ExternalInput")
    idx_d = nc.dram_tensor("idx16", [P, NIDX // 16], dt.int16, kind="ExternalInput")
    dst_d = nc.dram_tensor("dstv", [P, TT], dt.float32, kind="ExternalInput")
    nrm_d = nc.dram_tensor("nrmv", [P, TT], dt.float32, kind="ExternalInput")
    iota_d = nc.dram_tensor("iota", [P, P], dt.bfloat16, kind="ExternalInput")
    wih_d = {
        g: nc.dram_tensor(f"wih_{g}", [P, P], dt.bfloat16, kind="ExternalInput")
        for g in "igo"
    }
    bg_d = {
        g: nc.dram_tensor(f"bg_{g}", [P, 1], dt.float32, kind="ExternalInput")
        for g in "igo"
    }
    w1_d = nc.dram_tensor("w1", [P, P], dt.bfloat16, kind="ExternalInput")
    w2_d = nc.dram_tensor("w2", [P, P], dt.bfloat16, kind="ExternalInput")
    w3t_d = nc.dram_tensor("w3t", [P, P], dt.bfloat16, kind="ExternalInput")
    b1_d = nc.dram_tensor("b1", [P, 1], dt.float32, kind="ExternalInput")
    b2_d = nc.dram_tensor("b2", [P, 1], dt.float32, kind="ExternalInput")
    b3_d = nc.dram_tensor("b3", [P, 1], dt.float32, kind="ExternalInput")
    out_d = nc.dram_tensor("outT", [P, NPC], dt.float32, kind="ExternalOutput")

    bounce = [nc.dram_tensor(f"bounce{l}", [NPC, P], dt.bfloat16) for l in range(2)]
    table = [
        [nc.dram_tensor(f"table{l}_{q}", [CHROWS, P], dt.bfloat16) for q in range(CH)]
        for l in range(2)
    ]

    with tile.TileContext(nc) as tc, ExitStack() as ctx:
        konst = ctx.enter_context(tc.tile_pool(name="konst", bufs=1))
        big = ctx.enter_context(tc.tile_pool(name="big", bufs=1))

        def load_const(handle, shape, dtype):
            t = konst.tile(shape, dtype, tag=handle.name)
            nc.sync.dma_start(t[:], handle[:])
            return t

        iota_t = load_const(iota_d, [P, P], dt.bfloat16)
        wih_t = {g: load_const(wih_d[g], [P, P], dt.bfloat16) for g in "igo"}
        bg_t = {g: load_const(bg_d[g], [P, 1], dt.float32) for g in "igo"}
        w1_t = load_const(w1_d, [P, P], dt.bfloat16)
        w2_t = load_const(w2_d, [P, P], dt.bfloat16)
        w3t_t = load_const(w3t_d, [P, P], dt.bfloat16)
        b1_t = load_const(b1_d, [P, 1], dt.float32)
        b2_t = load_const(b2_d, [P, 1], dt.float32)
        b3_t = load_const(b3_d, [P, 1], dt.float32)
        idx_t = load_const(idx_d, [P, NIDX // 16], dt.int16)
        dst_t = load_const(dst_d, [P, TT], dt.float32)
        nrm_t = load_const(nrm_d, [P, TT], dt.float32)

        xT_t = big.tile([P, NPC], dt.bfloat16, tag="xT")  # x1T then x2T

        # ---------------- phase 1: LSTM -> hT (feature-major, bf16)
        with tc.tile_pool(name="h_pool", bufs=1) as hpool:
            hT_t = hpool.tile([P, NPC], dt.bfloat16, tag="hT")
            with (
                tc.tile_pool(name="lstm_sb", bufs=1) as lsb,
                tc.tile_pool(name="lstm_ps", bufs=6, space="PSUM") as lps,
                tc.tile_pool(name="lstm_tr", bufs=8) as ltr,
            ):
                zT_t = lsb.tile([P, NPC], dt.bfloat16, tag="zT")
                nc.sync.dma_start(zT_t[:], zT_d[:])

                nchunk = (NPC + LSTM_CHUNK - 1) // LSTM_CHUNK
                for k in range(nchunk):
                    c0 = k * LSTM_CHUNK
                    c1 = min(NPC, c0 + LSTM_CHUNK)
                    w = c1 - c0
                    gate = {}
                    for g in "igo":
                        ps = lps.tile([P, LSTM_CHUNK], dt.float32, tag="ps")
                        nc.tensor.matmul(
                            ps[:, :w], wih_t[g][:], zT_t[:, c0:c1], start=True, stop=True
                        )
                        fn = act.Tanh if g == "g" else act.Sigmoid
                        sg = ltr.tile([P, LSTM_CHUNK], dt.bfloat16, tag="sg" + g)
                        nc.scalar.activation(sg[:, :w], ps[:, :w], fn, bias=bg_t[g][:])
                        gate[g] = sg
                    ct = ltr.tile([P, LSTM_CHUNK], dt.bfloat16, tag="ct")
                    nc.vector.tensor_tensor(
                        ct[:, :w], gate["i"][:, :w], gate["g"][:, :w], op=alu.mult
                    )
                    th = ltr.tile([P, LSTM_CHUNK], dt.bfloat16, tag="th")
                    nc.scalar.activation(th[:, :w], ct[:, :w], act.Tanh)
                    nc.vector.tensor_tensor(
                        hT_t[:, c0:c1], gate["o"][:, :w], th[:, :w], op=alu.mult
                    )

            # ---------------- phase 2: m1 = (h @ W1) node-major -> bounce0
            _mm_to_bounce(nc, tc, hT_t, w1_t, bounce[0])

        _allgather(nc, bounce[0], table[0])

        with (
            tc.tile_pool(name="stag", bufs=6) as stag,
            tc.tile_pool(name="spool", bufs=12) as spool,
        ):
            # ------------- phase 3: edge layer 1 -> x1T = relu(agg + b1)
            def post1(b, nb, pa):
                nc.scalar.activation(
                    xT_t[:, b * P : b * P + nb], pa[:, :nb], act.Relu, bias=b1_t[:]
                )

            _edge_phase(nc, tc, table[0], sched, idx_t, dst_t, nrm_t, iota_t, stag, spool, post1)

            # ------------- phase 4: m2 = (x1 @ W2) node-major -> bounce1
            _mm_to_bounce(nc, tc, xT_t, w2_t, bounce[1])
            _allgather(nc, bounce[1], table[1])

            # ------------- phase 5: edge layer 2 -> x2T = agg + b2 (no relu)
            def post2(b, nb, pa):
                nc.vector.tensor_scalar(
                    xT_t[:, b * P : b * P + nb], pa[:, :nb], b2_t[:], None, op0=alu.add
                )

            _edge_phase(nc, tc, table[1], sched, idx_t, dst_t, nrm_t, iota_t, stag, spool, post2)

        # ---------------- phase 6: outT = relu(W3T.T @ x2T + b3)
        with (
            tc.tile_pool(name="out_ps", bufs=3, space="PSUM") as ops,
            tc.tile_pool(name="out_sb", bufs=3) as osb,
        ):
            nchunk = (NPC + LSTM_CHUNK - 1) // LSTM_CHUNK
            for k in range(nchunk):
                c0 = k * LSTM_CHUNK
                c1 = min(NPC, c0 + LSTM_CHUNK)
                w = c1 - c0
                ps = ops.tile([P, LSTM_CHUNK], dt.float32, tag="ps")
                nc.tensor.matmul(
                    ps[:, :w], w3t_t[:], xT_t[:, c0:c1], start=True, stop=True
                )
                ot = osb.tile([P, LSTM_CHUNK], dt.float32, tag="ot")
                nc.scalar.activation(ot[:, :w], ps[:, :w], act.Relu, bias=b3_t[:])
                nc.sync.dma_start(out_d[:, c0:c1], ot[:, :w])

    nc.compile()
    return nc


def _mm_to_bounce(nc, tc, featT, w_t, bounce_d):
    """Per 128-node block: matmul(lhsT=featT block, rhs=W) -> node-major
    [node, feat] psum -> bf16 stage -> one strided DMA into bounce DRAM."""
    dt = mybir.dt
    act = mybir.ActivationFunctionType
    with (
        tc.tile_pool(name="m_ps", bufs=2, space="PSUM") as mps,
        tc.tile_pool(name="m_sb", bufs=1) as msb,
    ):
        stage = msb.tile([P, NBLK * P], dt.bfloat16, tag="mstage")
        for b in range(NBLK):
            nb = min(P, NPC - b * P)
            pm = mps.tile([P, P], dt.float32, tag="pm")
            nc.tensor.matmul(
                pm[:nb, :], featT[:, b * P : b * P + nb], w_t[:], start=True, stop=True
            )
            nc.scalar.activation(
                stage[:nb, b * P : (b + 1) * P], pm[:nb, :], act.Copy
            )
        full = (NPC // P) * P  # 12416
        nc.sync.dma_start(
            bounce_d[:full, :].rearrange("(b p) f -> p b f", p=P),
            stage[:, : NPC // P * P].rearrange("p (b f) -> p b f", f=P),
        )
        rem = NPC - full
        if rem:
            nc.sync.dma_start(bounce_d[full:, :], stage[:rem, full:])


def _allgather(nc, bounce_d, tables_d):
    # one sub-AllGather per quarter: output q IS chunk table q (offset-free),
    # and chunk-q edge gathers can start as soon as AG#q lands.
    for q in range(CH):
        nc.gpsimd.collective_compute(
            "AllGather",
            mybir.AluOpType.bypass,
            replica_groups=[list(range(NCORES))],
            ins=[bounce_d[q * QROWS : (q + 1) * QROWS, :]],
            outs=[tables_d[q][:]],
        )


def _edge_phase(nc, tc, table_d, sched, idx_t, dst_t, nrm_t, iota_t, stag, spool, post):
    dt = mybir.dt
    alu = mybir.AluOpType
    T_run, base, ctb, pieces = (
        sched["T_run"],
        sched["base"],
        sched["ctb"],
        sched["pieces"],
    )
    piece_tiles = {}
    with tc.tile_pool(name="agg_ps", bufs=6, space="PSUM") as aps:
        for b in range(NBLK):
            nb = min(P, NPC - b * P)
            pa = aps.tile([P, P], dt.float32, tag="pa")
            ntile_b = int(T_run[:, b].sum())
            done = 0
            for chn in range(CH):
                for t in range(int(T_run[chn][b])):
                    gt = int(base[chn * NBLK + b]) + t
                    rel = gt - ctb[chn]
                    pi, slot = divmod(rel, GT)
                    key = (chn, pi)
                    if key not in piece_tiles:
                        pt0, pnt = pieces[chn][pi]
                        stg = stag.tile([P, GT, P], dt.bfloat16, tag="stag")
                        nc.gpsimd.dma_gather(
                            stg[:, :pnt, :],
                            table_d[chn][:],
                            idx_t[:, pt0 * 8 : (pt0 + pnt) * 8],
                            pnt * P,
                            pnt * P,
                            P,
                            single_packet=False,
                        )
                        piece_tiles[key] = stg
                    stg = piece_tiles[key]
                    st = spool.tile([P, P], dt.bfloat16, tag="st")
                    nc.vector.tensor_scalar(
                        st[:],
                        iota_t[:],
                        dst_t[:, gt : gt + 1],
                        nrm_t[:, gt : gt + 1],
                        op0=alu.is_equal,
                        op1=alu.mult,
                    )
                    nc.tensor.matmul(
                        pa[:],
                        stg[:, slot, :],
                        st[:],
                        start=(done == 0),
                        stop=(done == ntile_b - 1),
                    )
                    done += 1
            post(b, nb, pa)


# ---------------------------------------------------------------- entry


def build(z, edge_index, W_ih, W_hh, b_ih, b_hh, W1, b1, W2, b2, W3, b3):
    """Host prep + trace + compile. Returns (nc, in_maps)."""
    z = np.asarray(z, dtype=np.float32)
    W_ih = np.asarray(W_ih, dtype=np.float32)
    b = np.asarray(b_ih, dtype=np.float32) + np.asarray(b_hh, dtype=np.float32)

    arrs, sched = _prep_edges(edge_index)
    nc = _build_nc(sched)

    gi = {"i": 0, "g": 2, "o": 3}  # torch gate order i,f,g,o (f unused: c0=0)
    common = {
        "iota": np.ascontiguousarray(
            np.tile(np.arange(P, dtype=np.float32), (P, 1))
        ).astype(bf16),
        "w1": np.asarray(W1, np.float32).astype(bf16),
        "w2": np.asarray(W2, np.float32).astype(bf16),
        "w3t": np.ascontiguousarray(np.asarray(W3, np.float32).T).astype(bf16),
        "b1": np.asarray(b1, np.float32).reshape(P, 1).copy(),
        "b2": np.asarray(b2, np.float32).reshape(P, 1).copy(),
        "b3": np.asarray(b3, np.float32).reshape(P, 1).copy(),
    }
    for g, k in gi.items():
        common[f"wih_{g}"] = np.ascontiguousarray(
            W_ih[k * P : (k + 1) * P, :].T
        ).astype(bf16)
        common[f"bg_{g}"] = b[k * P : (k + 1) * P].reshape(P, 1).copy()

    in_maps = []
    for c in range(NCORES):
        idx16, dstt, nrmt = arrs[c]
        m = dict(common)
        m["zT"] = np.ascontiguousarray(z[c * NPC : (c + 1) * NPC].T).astype(bf16)
        m["idx16"] = idx16
        m["dstv"] = dstt
        m["nrmv"] = nrmt
        in_maps.append(m)
    return nc, in_maps


def assemble(results):
    out = np.empty((N, P), np.float32)
    for c in range(NCORES):
        out[c * NPC : (c + 1) * NPC] = results[c]["outT"].T
    return out


def kernel(z, edge_index, W_ih, W_hh, b_ih, b_hh, W1, b1, W2, b2, W3, b3):
    nc, in_maps = build(z, edge_index, W_ih, W_hh, b_ih, b_hh, W1, b1, W2, b2, W3, b3)
    res = run_bass_kernel_spmd(nc, in_maps, core_ids=list(range(NCORES)))
    return assemble(res.results)



# revision 9
# speedup vs baseline: 10.7587x; 10.7587x over previous
"""Trainium2 Bass kernel for nn_Decoder (GNN message passing):
LSTM(1 step) -> GCNConv -> ReLU -> GCNConv -> Linear -> ReLU on a
100K-node / 1.6M-edge graph, SPMD across 8 NeuronCores.

Strategy (dst-node sharding):
- Core c owns nodes [c*12500, (c+1)*12500) and all edges into them.
- Per-node compute (LSTM, x@W transforms) runs feature-major [128, nodes]
  so all matmuls need zero transposes and biases are per-partition.
- The GCN propagate gathers transformed rows from a bf16 node-major table
  in DRAM (built via AllGather of the 8 shards) with gpsimd.dma_gather,
  then scatter-adds via PE matmul with an on-chip selection matrix
  (tensor_scalar: iota==dst_idx -> * norm), accumulated in PSUM per
  128-dst block.
"""

from contextlib import ExitStack

import numpy as np
import ml_dtypes

import concourse.bacc as bacc
import concourse.mybir as mybir
import concourse.tile as tile
from concourse.bass_utils import run_bass_kernel_spmd

P = 128
N = 100000
NCORES = 8
NPC = N // NCORES            # 12500 nodes per core
NBLK = (NPC + P - 1) // P    # 98 dst blocks per core (last has 84)
CH = 4                       # src chunks (int16 gather index limit)
QROWS = NPC // CH            # 3125: per-rank quarter contributed to a chunk
CHROWS = QROWS * NCORES      # 25000 rows per chunk table
GT = 32                      # tiles (of 128 edges) per dma_gather
LSTM_CHUNK = 500             # nodes per LSTM/matmul column chunk

bf16 = ml_dtypes.bfloat16
f32 = np.float32


# ---------------------------------------------------------------- host prep


def _prep_edges(edge_index):
    """Sort/pad each core's incident edges into a cross-core-uniform tile
    schedule. Returns per-core device arrays + the static schedule."""
    src = np.asarray(edge_index[0], dtype=np.int64)
    dst = np.asarray(edge_index[1], dtype=np.int64)
    loops = np.arange(N, dtype=np.int64)
    src = np.concatenate([src, loops])
    dst = np.concatenate([dst, loops])

    deg = np.bincount(dst, minlength=N).astype(np.float64)
    dinv = 1.0 / np.sqrt(deg)
    norm = (dinv[src] * dinv[dst]).astype(np.float32)

    core_of = dst // NPC
    per_core = []
    counts = np.zeros((NCORES, CH, NBLK), np.int64)
    for c in range(NCORES):
        m = core_of == c
        s = src[m]
        d = dst[m] - c * NPC
        w = norm[m]
        ch = (s % NPC) // QROWS
        o = np.lexsort((d, ch))
        s, d, w, ch = s[o], d[o], w[o], ch[o]
        b = d // P
        counts[c] = np.bincount(ch * NBLK + b, minlength=CH * NBLK).reshape(
            CH, NBLK
        )
        per_core.append((s, d, w, ch, b))

    # tiles per (chunk, block) run: padded to the max across cores
    T_run = (counts.max(axis=0) + P - 1) // P          # [CH, NBLK]
    flat = T_run.reshape(-1)
    base = np.zeros(CH * NBLK + 1, np.int64)
    np.cumsum(flat, out=base[1:])                      # tile offset per run
    TT = int(base[-1])
    NIDX = TT * P
    ctb = [int(base[ch * NBLK]) for ch in range(CH)] + [TT]  # chunk tile base

    arrs = []
    for c in range(NCORES):
        s, d, w, ch, b = per_core[c]
        gid = ch * NBLK + b
        cnt = counts[c].reshape(-1)
        gstart = np.concatenate([[0], np.cumsum(cnt)[:-1]])
        within = np.arange(len(s)) - gstart[gid]
        pos = base[gid] * P + within

        idxs = np.zeros(NIDX, np.int16)                 # pad -> row 0 (valid)
        # chunk q table = concat over ranks of each rank's q-th quarter
        idxs[pos] = ((s // NPC) * QROWS + (s % QROWS)).astype(np.int16)
        dstv = np.full(NIDX, -1.0, np.float32)          # pad -> no dst match
        dstv[pos] = (d - b * P).astype(np.float32)
        nrmv = np.zeros(NIDX, np.float32)
        nrmv[pos] = w

        idx16 = np.tile(np.ascontiguousarray(idxs.reshape(-1, 16).T), (8, 1))
        dstt = np.ascontiguousarray(dstv.reshape(TT, P).T).astype(bf16)
        nrmt = np.ascontiguousarray(nrmv.reshape(TT, P).T).astype(bf16)
        arrs.append((idx16, dstt, nrmt))

    # gather pieces: per chunk, consecutive groups of <= GT tiles
    pieces = []
    for chn in range(CH):
        t0, t1 = ctb[chn], ctb[chn + 1]
        pieces.append([(t, min(GT, t1 - t)) for t in range(t0, t1, GT)])

    sched = dict(T_run=T_run, base=base, TT=TT, NIDX=NIDX, ctb=ctb, pieces=pieces)
    return arrs, sched


# ---------------------------------------------------------------- device


def _build_nc(sched):
    T_run, base, TT, NIDX, ctb, pieces = (
        sched["T_run"],
        sched["base"],
        sched["TT"],
        sched["NIDX"],
        sched["ctb"],
        sched["pieces"],
    )
    dt = mybir.dt
    alu = mybir.AluOpType
    act = mybir.ActivationFunctionType

    nc = bacc.Bacc(
        "TRN2",
        target_bir_lowering=False,
        debug=False,
        num_devices=NCORES,
        num_swdge_queues=4,
    )

    # ---- I/O
    zT_d = nc.dram_tensor("zT", [P, NPC], dt.bfloat16, kind="ExternalInput")
    idx_d = nc.dram_tensor("idx16", [P, NIDX // 16], dt.int16, kind="ExternalInput")
    dst_d = nc.dram_tensor("dstv", [P, TT], dt.bfloat16, kind="ExternalInput")
    nrm_d = nc.dram_tensor("nrmv", [P, TT], dt.bfloat16, kind="ExternalInput")
    iota_d = nc.dram_tensor("iota", [P, P], dt.bfloat16, kind="ExternalInput")
    wih_d = {
        g: nc.dram_tensor(f"wih_{g}", [P, P], dt.bfloat16, kind="ExternalInput")
        for g in "igo"
    }
    bg_d = {
        g: nc.dram_tensor(f"bg_{g}", [P, 1], dt.float32, kind="ExternalInput")
        for g in "igo"
    }
    w1_d = nc.dram_tensor("w1", [P, P], dt.bfloat16, kind="ExternalInput")
    w2_d = nc.dram_tensor("w2", [P, P], dt.bfloat16, kind="ExternalInput")
    w3t_d = nc.dram_tensor("w3t", [P, P], dt.bfloat16, kind="ExternalInput")
    b1_d = nc.dram_tensor("b1", [P, 1], dt.float32, kind="ExternalInput")
    b2_d = nc.dram_tensor("b2", [P, 1], dt.float32, kind="ExternalInput")
    b3_d = nc.dram_tensor("b3", [P, 1], dt.float32, kind="ExternalInput")
    out_d = nc.dram_tensor("outT", [P, NPC], dt.float32, kind="ExternalOutput")

    bounce = [nc.dram_tensor(f"bounce{l}", [NPC, P], dt.bfloat16) for l in range(2)]
    table = [
        [nc.dram_tensor(f"table{l}_{q}", [CHROWS, P], dt.bfloat16) for q in range(CH)]
        for l in range(2)
    ]

    with tile.TileContext(nc) as tc, ExitStack() as ctx:
        konst = ctx.enter_context(tc.tile_pool(name="konst", bufs=1))
        big = ctx.enter_context(tc.tile_pool(name="big", bufs=1))

        def load_const(handle, shape, dtype):
            t = konst.tile(shape, dtype, tag=handle.name)
            nc.sync.dma_start(t[:], handle[:])
            return t

        iota_t = load_const(iota_d, [P, P], dt.bfloat16)
        wih_t = {g: load_const(wih_d[g], [P, P], dt.bfloat16) for g in "igo"}
        bg_t = {g: load_const(bg_d[g], [P, 1], dt.float32) for g in "igo"}
        w1_t = load_const(w1_d, [P, P], dt.bfloat16)
        w2_t = load_const(w2_d, [P, P], dt.bfloat16)
        w3t_t = load_const(w3t_d, [P, P], dt.bfloat16)
        b1_t = load_const(b1_d, [P, 1], dt.float32)
        b2_t = load_const(b2_d, [P, 1], dt.float32)
        b3_t = load_const(b3_d, [P, 1], dt.float32)
        idx_t = load_const(idx_d, [P, NIDX // 16], dt.int16)
        dst_t = load_const(dst_d, [P, TT], dt.bfloat16)
        nrm_t = load_const(nrm_d, [P, TT], dt.bfloat16)

        xT_t = big.tile([P, NPC], dt.bfloat16, tag="xT")  # x1T then x2T

        # ---------------- phase 1: LSTM -> hT (feature-major, bf16)
        with tc.tile_pool(name="h_pool", bufs=1) as hpool:
            hT_t = hpool.tile([P, NPC], dt.bfloat16, tag="hT")
            with (
                tc.tile_pool(name="lstm_sb", bufs=1) as lsb,
                tc.tile_pool(name="lstm_ps", bufs=6, space="PSUM") as lps,
                tc.tile_pool(name="lstm_tr", bufs=8) as ltr,
            ):
                zT_t = lsb.tile([P, NPC], dt.bfloat16, tag="zT")
                nc.sync.dma_start(zT_t[:], zT_d[:])

                nchunk = (NPC + LSTM_CHUNK - 1) // LSTM_CHUNK
                for k in range(nchunk):
                    c0 = k * LSTM_CHUNK
                    c1 = min(NPC, c0 + LSTM_CHUNK)
                    w = c1 - c0
                    gate = {}
                    for g in "igo":
                        ps = lps.tile([P, LSTM_CHUNK], dt.float32, tag="ps")
                        nc.tensor.matmul(
                            ps[:, :w], wih_t[g][:], zT_t[:, c0:c1], start=True, stop=True
                        )
                        fn = act.Tanh if g == "g" else act.Sigmoid
                        sg = ltr.tile([P, LSTM_CHUNK], dt.bfloat16, tag="sg" + g)
                        nc.scalar.activation(sg[:, :w], ps[:, :w], fn, bias=bg_t[g][:])
                        gate[g] = sg
                    ct = ltr.tile([P, LSTM_CHUNK], dt.bfloat16, tag="ct")
                    nc.vector.tensor_tensor(
                        ct[:, :w], gate["i"][:, :w], gate["g"][:, :w], op=alu.mult
                    )
                    th = ltr.tile([P, LSTM_CHUNK], dt.bfloat16, tag="th")
                    nc.scalar.activation(th[:, :w], ct[:, :w], act.Tanh)
                    nc.vector.tensor_tensor(
                        hT_t[:, c0:c1], gate["o"][:, :w], th[:, :w], op=alu.mult
                    )

            # ---------------- phase 2: m1 = (h @ W1) node-major -> bounce0
            _mm_to_bounce(nc, tc, hT_t, w1_t, bounce[0])

        _allgather(nc, bounce[0], table[0])

        with (
            tc.tile_pool(name="stag", bufs=6) as stag,
            tc.tile_pool(name="spool", bufs=4) as spool,
        ):
            # ------------- phase 3: edge layer 1 -> x1T = relu(agg + b1)
            def post1(b, nb, pa):
                nc.scalar.activation(
                    xT_t[:, b * P : b * P + nb], pa[:, :nb], act.Relu, bias=b1_t[:]
                )

            _edge_phase(nc, tc, table[0], sched, idx_t, dst_t, nrm_t, iota_t, stag, spool, post1)

            # ------------- phase 4: m2 = (x1 @ W2) node-major -> bounce1
            _mm_to_bounce(nc, tc, xT_t, w2_t, bounce[1])
            _allgather(nc, bounce[1], table[1])

            # ------------- phase 5: edge layer 2 -> x2T = agg + b2 (no relu)
            def post2(b, nb, pa):
                nc.vector.tensor_scalar(
                    xT_t[:, b * P : b * P + nb], pa[:, :nb], b2_t[:], None, op0=alu.add
                )

            _edge_phase(nc, tc, table[1], sched, idx_t, dst_t, nrm_t, iota_t, stag, spool, post2)

        # ---------------- phase 6: outT = relu(W3T.T @ x2T + b3)
        with (
            tc.tile_pool(name="out_ps", bufs=3, space="PSUM") as ops,
            tc.tile_pool(name="out_sb", bufs=3) as osb,
        ):
            nchunk = (NPC + LSTM_CHUNK - 1) // LSTM_CHUNK
            for k in range(nchunk):
                c0 = k * LSTM_CHUNK
                c1 = min(NPC, c0 + LSTM_CHUNK)
                w = c1 - c0
                ps = ops.tile([P, LSTM_CHUNK], dt.float32, tag="ps")
                nc.tensor.matmul(
                    ps[:, :w], w3t_t[:], xT_t[:, c0:c1], start=True, stop=True
                )
                ot = osb.tile([P, LSTM_CHUNK], dt.float32, tag="ot")
                nc.scalar.activation(ot[:, :w], ps[:, :w], act.Relu, bias=b3_t[:])
                nc.sync.dma_start(out_d[:, c0:c1], ot[:, :w])

    nc.compile()
    return nc


def _mm_to_bounce(nc, tc, featT, w_t, bounce_d):
    """Per 128-node block: matmul(lhsT=featT block, rhs=W) -> node-major
    [node, feat] psum -> bf16 stage -> one strided DMA into bounce DRAM."""
    dt = mybir.dt
    act = mybir.ActivationFunctionType
    with (
        tc.tile_pool(name="m_ps", bufs=2, space="PSUM") as mps,
        tc.tile_pool(name="m_sb", bufs=1) as msb,
    ):
        stage = msb.tile([P, NBLK * P], dt.bfloat16, tag="mstage")
        for b in range(NBLK):
            nb = min(P, NPC - b * P)
            pm = mps.tile([P, P], dt.float32, tag="pm")
            nc.tensor.matmul(
                pm[:nb, :], featT[:, b * P : b * P + nb], w_t[:], start=True, stop=True
            )
            nc.scalar.activation(
                stage[:nb, b * P : (b + 1) * P], pm[:nb, :], act.Copy
            )
        full = (NPC // P) * P  # 12416
        nc.sync.dma_start(
            bounce_d[:full, :].rearrange("(b p) f -> p b f", p=P),
            stage[:, : NPC // P * P].rearrange("p (b f) -> p b f", f=P),
        )
        rem = NPC - full
        if rem:
            nc.sync.dma_start(bounce_d[full:, :], stage[:rem, full:])


def _allgather(nc, bounce_d, tables_d):
    # one sub-AllGather per quarter: output q IS chunk table q (offset-free),
    # and chunk-q edge gathers can start as soon as AG#q lands.
    for q in range(CH):
        nc.gpsimd.collective_compute(
            "AllGather",
            mybir.AluOpType.bypass,
            replica_groups=[list(range(NCORES))],
            ins=[bounce_d[q * QROWS : (q + 1) * QROWS, :]],
            outs=[tables_d[q][:]],
        )


def _edge_phase(nc, tc, table_d, sched, idx_t, dst_t, nrm_t, iota_t, stag, spool, post):
    dt = mybir.dt
    alu = mybir.AluOpType
    T_run, base, ctb, pieces = (
        sched["T_run"],
        sched["base"],
        sched["ctb"],
        sched["pieces"],
    )
    piece_tiles = {}
    sel_tiles = {}
    with tc.tile_pool(name="agg_ps", bufs=6, space="PSUM") as aps:
        for b in range(NBLK):
            nb = min(P, NPC - b * P)
            pa = aps.tile([P, P], dt.float32, tag="pa")
            ntile_b = int(T_run[:, b].sum())
            done = 0
            for chn in range(CH):
                for t in range(int(T_run[chn][b])):
                    gt = int(base[chn * NBLK + b]) + t
                    rel = gt - ctb[chn]
                    pi, slot = divmod(rel, GT)
                    key = (chn, pi)
                    if key not in piece_tiles:
                        pt0, pnt = pieces[chn][pi]
                        stg = stag.tile([P, GT, P], dt.bfloat16, tag="stag")
                        nc.gpsimd.dma_gather(
                            stg[:, :pnt, :],
                            table_d[chn][:],
                            idx_t[:, pt0 * 8 : (pt0 + pnt) * 8],
                            pnt * P,
                            pnt * P,
                            P,
                            single_packet=False,
                            queue_num=chn,
                        )
                        piece_tiles[key] = stg
                        # batched select build: sel[e, t', j] =
                        #   (iota[e, j] == dst[e, pt0+t']) * nrm[e, pt0+t']
                        sel = spool.tile([P, GT, P], dt.bfloat16, tag="sel")
                        nc.vector.tensor_tensor(
                            sel[:, :pnt, :],
                            iota_t[:].unsqueeze(1).broadcast_to([P, pnt, P]),
                            dst_t[:, pt0 : pt0 + pnt]
                            .unsqueeze(2)
                            .broadcast_to([P, pnt, P]),
                            op=alu.is_equal,
                        )
                        nc.vector.tensor_tensor(
                            sel[:, :pnt, :],
                            sel[:, :pnt, :],
                            nrm_t[:, pt0 : pt0 + pnt]
                            .unsqueeze(2)
                            .broadcast_to([P, pnt, P]),
                            op=alu.mult,
                        )
                        sel_tiles[key] = sel
                    stg = piece_tiles[key]
                    sel = sel_tiles[key]
                    nc.tensor.matmul(
                        pa[:],
                        stg[:, slot, :],
                        sel[:, slot, :],
                        start=(done == 0),
                        stop=(done == ntile_b - 1),
                    )
                    done += 1
            post(b, nb, pa)


# ---------------------------------------------------------------- entry


def build(z, edge_index, W_ih, W_hh, b_ih, b_hh, W1, b1, W2, b2, W3, b3):
    """Host prep + trace + compile. Returns (nc, in_maps)."""
    z = np.asarray(z, dtype=np.float32)
    W_ih = np.asarray(W_ih, dtype=np.float32)
    b = np.asarray(b_ih, dtype=np.float32) + np.asarray(b_hh, dtype=np.float32)

    arrs, sched = _prep_edges(edge_index)
    nc = _build_nc(sched)

    gi = {"i": 0, "g": 2, "o": 3}  # torch gate order i,f,g,o (f unused: c0=0)
    common = {
        "iota": np.ascontiguousarray(
            np.tile(np.arange(P, dtype=np.float32), (P, 1))
        ).astype(bf16),
        "w1": np.asarray(W1, np.float32).astype(bf16),
        "w2": np.asarray(W2, np.float32).astype(bf16),
        "w3t": np.ascontiguousarray(np.asarray(W3, np.float32).T).astype(bf16),
        "b1": np.asarray(b1, np.float32).reshape(P, 1).copy(),
        "b2": np.asarray(b2, np.float32).reshape(P, 1).copy(),
        "b3": np.asarray(b3, np.float32).reshape(P, 1).copy(),
    }
    for g, k in gi.items():
        common[f"wih_{g}"] = np.ascontiguousarray(
            W_ih[k * P : (k + 1) * P, :].T
        ).astype(bf16)
        common[f"bg_{g}"] = b[k * P : (k + 1) * P].reshape(P, 1).copy()

    in_maps = []
    for c in range(NCORES):
        idx16, dstt, nrmt = arrs[c]
        m = dict(common)
        m["zT"] = np.ascontiguousarray(z[c * NPC : (c + 1) * NPC].T).astype(bf16)
        m["idx16"] = idx16
        m["dstv"] = dstt
        m["nrmv"] = nrmt
        in_maps.append(m)
    return nc, in_maps


def assemble(results):
    out = np.empty((N, P), np.float32)
    for c in range(NCORES):
        out[c * NPC : (c + 1) * NPC] = results[c]["outT"].T
    return out


def kernel(z, edge_index, W_ih, W_hh, b_ih, b_hh, W1, b1, W2, b2, W3, b3):
    nc, in_maps = build(z, edge_index, W_ih, W_hh, b_ih, b_hh, W1, b1, W2, b2, W3, b3)
    res = run_bass_kernel_spmd(nc, in_maps, core_ids=list(range(NCORES)))
    return assemble(res.results)



# revision 31
# speedup vs baseline: 12.5784x; 1.1691x over previous
"""Trainium2 Bass kernel for nn_Decoder (GNN message passing):
LSTM(1 step) -> GCNConv -> ReLU -> GCNConv -> Linear -> ReLU on a
100K-node / 1.6M-edge graph, SPMD across 8 NeuronCores.

Strategy (dst-node sharding):
- Core c owns nodes [c*12500, (c+1)*12500) and all edges into them.
- Per-node compute (LSTM, x@W transforms) runs feature-major [128, nodes]
  so all matmuls need zero transposes and biases are per-partition.
- The GCN propagate gathers transformed rows from a bf16 node-major table
  in DRAM (built via sub-AllGathers of the 8 shards, pipelined against the
  per-block transform matmuls) with gpsimd.dma_gather spread over 4 SWDGE
  queues (each queue runs on its own Q7 core pair), then scatter-adds via
  PE matmul with an on-chip selection matrix built in batched broadcast
  tensor_tensor ops (iota==dst -> * norm), accumulated in PSUM.
- Edge tiles are ordered group-major (groups of 8 dst blocks, chunk-major
  inside) so chunk-q gathers only wait on sub-AllGather q, and 8 PSUM
  accumulators carry a group across the 4 chunk sweeps.
"""

from contextlib import ExitStack

import numpy as np
import ml_dtypes

import concourse.bacc as bacc
import concourse.mybir as mybir
import concourse.tile as tile
from concourse.bass_utils import run_bass_kernel_spmd

P = 128
N = 100000
NCORES = 8
NPC = N // NCORES            # 12500 nodes per core
NBLK = (NPC + P - 1) // P    # 98 dst blocks per core (last has 84)
CH = 4                       # src chunks (int16 gather index limit)
QROWS = NPC // CH            # 3125: per-rank quarter contributed to a chunk
CHROWS = QROWS * NCORES      # 25000 rows per chunk table
GT = 24                      # max tiles (of 128 edges) per dma_gather
GRP = 6                      # dst blocks per PSUM accumulation group (6 PSUM
                             # banks for accumulators + 2 for the transform)
NGRP = (NBLK + GRP - 1) // GRP
MMB = 25                     # transform/bounce blocks per sub-AllGather piece
LSTM_CHUNK = 500             # nodes per LSTM/matmul column chunk

bf16 = ml_dtypes.bfloat16
f32 = np.float32


# ---------------------------------------------------------------- host prep


def _prep_edges(edge_index):
    """Sort/pad each core's incident edges into a cross-core-uniform tile
    schedule, ordered group-major: (dst-block group, src chunk, block, tile).
    Returns per-core device arrays + the static schedule."""
    src = np.asarray(edge_index[0], dtype=np.int64)
    dst = np.asarray(edge_index[1], dtype=np.int64)

    # self-loops enter deg/norm but are applied on-chip from the transform
    # stage (diag matmul), not gathered
    deg = np.bincount(dst, minlength=N).astype(np.float64) + 1.0
    dinv = 1.0 / np.sqrt(deg)
    norm = (dinv[src] * dinv[dst]).astype(np.float32)
    dinv2 = (dinv * dinv).astype(np.float32)

    core_of = dst // NPC
    per_core = []
    counts = np.zeros((NCORES, CH, NBLK), np.int64)
    for c in range(NCORES):
        m = core_of == c
        s = src[m]
        d = dst[m] - c * NPC
        w = norm[m]
        ch = (s % NPC) // QROWS
        o = np.lexsort((d, ch))
        s, d, w, ch = s[o], d[o], w[o], ch[o]
        b = d // P
        counts[c] = np.bincount(ch * NBLK + b, minlength=CH * NBLK).reshape(
            CH, NBLK
        )
        per_core.append((s, d, w, ch, b))

    # tiles per (chunk, block) run: padded to the max across cores
    T_run = (counts.max(axis=0) + P - 1) // P          # [CH, NBLK]

    # group-major tile order: for g: for ch: for b in group
    run_base = np.zeros((CH, NBLK), np.int64)
    t = 0
    seg = {}                                           # (g, ch) -> (t0, ntiles)
    for g in range(NGRP):
        blo, bhi = g * GRP, min(NBLK, (g + 1) * GRP)
        for ch in range(CH):
            t0 = t
            for b in range(blo, bhi):
                run_base[ch, b] = t
                t += int(T_run[ch, b])
            seg[(g, ch)] = (t0, t - t0)
    TT = int(t)
    NIDX = TT * P

    # gather pieces: per (g, ch) segment, consecutive groups of <= GT tiles
    pieces = {}
    tile_piece = {}                                    # gt -> (g, ch, pi, slot)
    for (g, ch), (t0, nt) in seg.items():
        plist = [(t0 + k, min(GT, nt - k)) for k in range(0, nt, GT)]
        pieces[(g, ch)] = plist
        for pi, (p0, pn) in enumerate(plist):
            for sl in range(pn):
                tile_piece[p0 + sl] = (g, ch, pi, sl)

    arrs = []
    for c in range(NCORES):
        s, d, w, ch, b = per_core[c]
        gid = ch * NBLK + b
        cnt = counts[c].reshape(-1)
        gstart = np.concatenate([[0], np.cumsum(cnt)[:-1]])
        within = np.arange(len(s)) - gstart[gid]
        pos = run_base[ch, b] * P + within

        idxs = np.zeros(NIDX, np.int16)                 # pad -> row 0 (valid)
        # chunk q table = concat over ranks of each rank's q-th quarter
        idxs[pos] = ((s // NPC) * QROWS + (s % QROWS)).astype(np.int16)
        dstv = np.full(NIDX, -1.0, np.float32)          # pad -> no dst match
        dstv[pos] = (d - b * P).astype(np.float32)
        nrmv = np.zeros(NIDX, np.float32)
        nrmv[pos] = w

        idx16 = np.tile(np.ascontiguousarray(idxs.reshape(-1, 16).T), (8, 1))
        dstt = np.ascontiguousarray(dstv.reshape(TT, P).T).astype(bf16)
        nrmt = np.ascontiguousarray(nrmv.reshape(TT, P).T).astype(bf16)
        d2 = np.zeros((NBLK, P), np.float32)          # [block, node-in-block]
        d2.reshape(-1)[:NPC] = dinv2[c * NPC : (c + 1) * NPC]
        d2t = np.ascontiguousarray(d2.T).astype(bf16)  # [P, NBLK]
        arrs.append((idx16, dstt, nrmt, d2t))

    sched = dict(
        T_run=T_run, run_base=run_base, TT=TT, NIDX=NIDX,
        pieces=pieces, tile_piece=tile_piece,
    )
    return arrs, sched


# ---------------------------------------------------------------- device


def _build_nc(sched):
    TT = sched["TT"]
    NIDX = sched["NIDX"]
    dt = mybir.dt
    alu = mybir.AluOpType
    act = mybir.ActivationFunctionType

    nc = bacc.Bacc(
        "TRN2",
        target_bir_lowering=False,
        debug=False,
        num_devices=NCORES,
        num_swdge_queues=4,
    )

    # ---- I/O
    zT_d = nc.dram_tensor("zT", [P, NPC], dt.bfloat16, kind="ExternalInput")
    idx_d = nc.dram_tensor("idx16", [P, NIDX // 16], dt.int16, kind="ExternalInput")
    dst_d = nc.dram_tensor("dstv", [P, TT], dt.bfloat16, kind="ExternalInput")
    nrm_d = nc.dram_tensor("nrmv", [P, TT], dt.bfloat16, kind="ExternalInput")
    iota_d = nc.dram_tensor("iota", [P, P], dt.bfloat16, kind="ExternalInput")
    ident_d = nc.dram_tensor("ident", [P, P], dt.bfloat16, kind="ExternalInput")
    dinv2_d = nc.dram_tensor("dinv2", [P, NBLK], dt.bfloat16, kind="ExternalInput")
    wih_d = {
        g: nc.dram_tensor(f"wih_{g}", [P, P], dt.bfloat16, kind="ExternalInput")
        for g in "igo"
    }
    bg_d = {
        g: nc.dram_tensor(f"bg_{g}", [P, 1], dt.float32, kind="ExternalInput")
        for g in "igo"
    }
    w1_d = nc.dram_tensor("w1", [P, P], dt.bfloat16, kind="ExternalInput")
    w2_d = nc.dram_tensor("w2", [P, P], dt.bfloat16, kind="ExternalInput")
    w3t_d = nc.dram_tensor("w3t", [P, P], dt.bfloat16, kind="ExternalInput")
    b1_d = nc.dram_tensor("b1", [P, 1], dt.float32, kind="ExternalInput")
    b2_d = nc.dram_tensor("b2", [P, 1], dt.float32, kind="ExternalInput")
    b3_d = nc.dram_tensor("b3", [P, 1], dt.float32, kind="ExternalInput")
    out_d = nc.dram_tensor("outT", [P, NPC], dt.float32, kind="ExternalOutput")

    bounce = [nc.dram_tensor(f"bounce{l}", [NPC, P], dt.bfloat16) for l in range(2)]
    table = [
        [nc.dram_tensor(f"table{l}_{q}", [CHROWS, P], dt.bfloat16) for q in range(CH)]
        for l in range(2)
    ]

    with tile.TileContext(nc) as tc, ExitStack() as ctx:
        konst = ctx.enter_context(tc.tile_pool(name="konst", bufs=1))
        big = ctx.enter_context(tc.tile_pool(name="big", bufs=1))

        def load_const(handle, shape, dtype):
            t = konst.tile(shape, dtype, tag=handle.name)
            nc.sync.dma_start(t[:], handle[:])
            return t

        iota_t = load_const(iota_d, [P, P], dt.bfloat16)
        ident_t = load_const(ident_d, [P, P], dt.bfloat16)
        dinv2_t = load_const(dinv2_d, [P, NBLK], dt.bfloat16)
        wih_t = {g: load_const(wih_d[g], [P, P], dt.bfloat16) for g in "igo"}
        bg_t = {g: load_const(bg_d[g], [P, 1], dt.float32) for g in "igo"}
        w1_t = load_const(w1_d, [P, P], dt.bfloat16)
        w2_t = load_const(w2_d, [P, P], dt.bfloat16)
        w3t_t = load_const(w3t_d, [P, P], dt.bfloat16)
        b1_t = load_const(b1_d, [P, 1], dt.float32)
        b2_t = load_const(b2_d, [P, 1], dt.float32)
        b3_t = load_const(b3_d, [P, 1], dt.float32)
        idx_t = load_const(idx_d, [P, NIDX // 16], dt.int16)
        dst_t = load_const(dst_d, [P, TT], dt.bfloat16)
        nrm_t = load_const(nrm_d, [P, TT], dt.bfloat16)

        xT_t = big.tile([P, NPC], dt.bfloat16, tag="xT")  # x1T then x2T

        mm_ps = ctx.enter_context(tc.tile_pool(name="m_ps", bufs=2, space="PSUM"))
        mm_sb = ctx.enter_context(tc.tile_pool(name="m_sb", bufs=1))

        # ---------------- phase 1: LSTM -> hT (feature-major, bf16)
        with tc.tile_pool(name="h_pool", bufs=1) as hpool:
            hT_t = hpool.tile([P, NPC], dt.bfloat16, tag="hT")
            with (
                tc.tile_pool(name="lstm_sb", bufs=1) as lsb,
                tc.tile_pool(name="lstm_ps", bufs=6, space="PSUM") as lps,
                tc.tile_pool(name="lstm_tr", bufs=8) as ltr,
            ):
                zT_t = lsb.tile([P, NPC], dt.bfloat16, tag="zT")
                nc.sync.dma_start(zT_t[:], zT_d[:])

                pipe1 = _MMPipe(
                    nc, tc, mm_ps, mm_sb, hT_t, w1_t, bounce[0], table[0]
                )
                nchunk = (NPC + LSTM_CHUNK - 1) // LSTM_CHUNK
                for k in range(nchunk):
                    c0 = k * LSTM_CHUNK
                    c1 = min(NPC, c0 + LSTM_CHUNK)
                    w = c1 - c0
                    gate = {}
                    for g in "igo":
                        ps = lps.tile([P, LSTM_CHUNK], dt.float32, tag="ps")
                        nc.tensor.matmul(
                            ps[:, :w], wih_t[g][:], zT_t[:, c0:c1], start=True, stop=True
                        )
                        fn = act.Tanh if g == "g" else act.Sigmoid
                        sg = ltr.tile([P, LSTM_CHUNK], dt.bfloat16, tag="sg" + g)
                        nc.scalar.activation(sg[:, :w], ps[:, :w], fn, bias=bg_t[g][:])
                        gate[g] = sg
                    ct = ltr.tile([P, LSTM_CHUNK], dt.bfloat16, tag="ct")
                    nc.vector.tensor_tensor(
                        ct[:, :w], gate["i"][:, :w], gate["g"][:, :w], op=alu.mult
                    )
                    th = ltr.tile([P, LSTM_CHUNK], dt.bfloat16, tag="th")
                    nc.scalar.activation(th[:, :w], ct[:, :w], act.Tanh)
                    nc.vector.tensor_tensor(
                        hT_t[:, c0:c1], gate["o"][:, :w], th[:, :w], op=alu.mult
                    )
                    # phase 2 interleaved: transform blocks fully covered by
                    # the LSTM so sub-AllGathers start during the LSTM sweep
                    pipe1.advance(c1 // P)

            pipe1.advance(NBLK)

        with (
            tc.tile_pool(name="stag", bufs=7) as stag,
            tc.tile_pool(name="spool", bufs=5) as spool,
        ):
            # ------------- phase 3: edge layer 1 -> x1T = relu(agg + b1),
            # interleaved with phase 4 (m2 = x1 @ W2 -> bounce1 + AGs)
            def post1(b, nb, pa):
                nc.scalar.activation(
                    xT_t[:, b * P : b * P + nb], pa[:, :nb], act.Relu, bias=b1_t[:]
                )

            pipe2 = _MMPipe(nc, tc, mm_ps, mm_sb, xT_t, w2_t, bounce[1], table[1])
            _edge_phase(
                nc, tc, table[0], sched, idx_t, dst_t, nrm_t, iota_t,
                stag, spool, post1, pipe1.stage, dinv2_t, ident_t,
                after_group=pipe2.advance,
            )
            pipe2.advance(NBLK)

            # ------------- phase 5: edge layer 2 -> x2T = agg + b2 (no relu)
            def post2(b, nb, pa):
                nc.vector.tensor_scalar(
                    xT_t[:, b * P : b * P + nb], pa[:, :nb], b2_t[:], None, op0=alu.add
                )

            _edge_phase(
                nc, tc, table[1], sched, idx_t, dst_t, nrm_t, iota_t,
                stag, spool, post2, pipe2.stage, dinv2_t, ident_t,
            )

        # ---------------- phase 6: outT = relu(W3T.T @ x2T + b3)
        with (
            tc.tile_pool(name="out_ps", bufs=3, space="PSUM") as ops,
            tc.tile_pool(name="out_sb", bufs=3) as osb,
        ):
            nchunk = (NPC + LSTM_CHUNK - 1) // LSTM_CHUNK
            for k in range(nchunk):
                c0 = k * LSTM_CHUNK
                c1 = min(NPC, c0 + LSTM_CHUNK)
                w = c1 - c0
                ps = ops.tile([P, LSTM_CHUNK], dt.float32, tag="ps")
                nc.tensor.matmul(
                    ps[:, :w], w3t_t[:], xT_t[:, c0:c1], start=True, stop=True
                )
                ot = osb.tile([P, LSTM_CHUNK], dt.float32, tag="ot")
                nc.scalar.activation(ot[:, :w], ps[:, :w], act.Relu, bias=b3_t[:])
                nc.sync.dma_start(out_d[:, c0:c1], ot[:, :w])

    nc.compile()
    return nc


class _MMPipe:
    """Per-block transform (featT block @ W -> node-major bf16 stage),
    with bounce-DMA + sub-AllGather emitted per MMB-block piece so the
    collectives overlap trailing compute."""

    def __init__(self, nc, tc, mm_ps, mm_sb, featT, w_t, bounce_d, tables_d):
        self.nc = nc
        self.mm_ps = mm_ps
        self.mm_sb = mm_sb
        self.featT = featT
        self.w_t = w_t
        self.bounce_d = bounce_d
        self.tables_d = tables_d
        # per-pipe stage: edge phase l reads pipe l's stage for the self-loop
        # diag matmuls, so the two layers' stages must coexist
        self.stage = mm_sb.tile(
            [P, NBLK * P], mybir.dt.bfloat16,
            tag=f"mst_{bounce_d.name}", name=f"stage_{bounce_d.name}",
        )
        self.done_b = 0
        self.piece = 0

    def advance(self, bend):
        nc = self.nc
        dt = mybir.dt
        act = mybir.ActivationFunctionType
        for b in range(self.done_b, bend):
            nb = min(P, NPC - b * P)
            pm = self.mm_ps.tile([P, P], dt.float32, tag="pm")
            nc.tensor.matmul(
                pm[:nb, :],
                self.featT[:, b * P : b * P + nb],
                self.w_t[:],
                start=True,
                stop=True,
            )
            nc.scalar.activation(
                self.stage[:nb, b * P : (b + 1) * P], pm[:nb, :], act.Copy
            )
        self.done_b = bend

        while self.piece < CH and (
            (self.piece + 1) * MMB <= bend or bend == NBLK
        ):
            p = self.piece
            blo = p * MMB
            bhi = min(NBLK, blo + MMB)
            full = min(bhi * P, (NPC // P) * P)
            nc.sync.dma_start(
                self.bounce_d[blo * P : full, :].rearrange(
                    "(b p) f -> p b f", p=P
                ),
                self.stage[:, blo * P : full].rearrange("p (b f) -> p b f", f=P),
            )
            if bhi * P > full:          # tail remainder rows (12416..12500)
                rem = NPC - full
                nc.sync.dma_start(
                    self.bounce_d[full:, :], self.stage[:rem, full:]
                )
            nc.gpsimd.collective_compute(
                "AllGather",
                mybir.AluOpType.bypass,
                replica_groups=[list(range(NCORES))],
                ins=[self.bounce_d[p * QROWS : (p + 1) * QROWS, :]],
                outs=[self.tables_d[p][:]],
            )
            self.piece += 1


def _edge_phase(
    nc, tc, table_d, sched, idx_t, dst_t, nrm_t, iota_t, stag, spool, post,
    stage, dinv2_t, ident_t, after_group=None,
):
    dt = mybir.dt
    alu = mybir.AluOpType
    T_run = sched["T_run"]
    run_base = sched["run_base"]
    pieces = sched["pieces"]
    tile_piece = sched["tile_piece"]

    last_ch = {}
    for b in range(NBLK):
        nz = [ch for ch in range(CH) if T_run[ch][b] > 0]
        last_ch[b] = nz[-1] if nz else -1

    piece_tiles = {}
    sel_tiles = {}
    with tc.tile_pool(name="agg_ps", bufs=1, space="PSUM") as aps:
        for g in range(NGRP):
            blo, bhi = g * GRP, min(NBLK, (g + 1) * GRP)
            ng = bhi - blo
            pa = {}
            for b in range(blo, bhi):
                pa_b = aps.tile(
                    [P, P], dt.float32, tag=f"pa{b - blo}", name=f"pa_{g}_{b}"
                )
                pa[b] = pa_b
            # self-loops: pa[b] starts as stage[b]^T * dinv2 (diag select)
            sdg = spool.tile(
                [P, GRP, P], dt.bfloat16, tag="sdg", bufs=2, name="sdg"
            )
            nc.vector.tensor_tensor(
                sdg[:, :ng, :],
                ident_t[:].unsqueeze(1).broadcast_to([P, ng, P]),
                dinv2_t[:, blo:bhi].unsqueeze(2).broadcast_to([P, ng, P]),
                op=alu.mult,
            )
            for b in range(blo, bhi):
                nb = min(P, NPC - b * P)
                nc.tensor.matmul(
                    pa[b][:],
                    stage[:nb, b * P : (b + 1) * P],
                    sdg[:nb, b - blo, :],
                    start=True,
                    stop=(last_ch[b] < 0),
                )
            for ch in range(CH):
                for b in range(blo, bhi):
                    for t in range(int(T_run[ch][b])):
                        gt = int(run_base[ch][b]) + t
                        gg, cc, pi, slot = tile_piece[gt]
                        key = (gg, cc, pi)
                        if key not in piece_tiles:
                            pt0, pnt = pieces[(gg, cc)][pi]
                            stg = stag.tile([P, GT, P], dt.bfloat16, tag="stag")
                            nc.gpsimd.dma_gather(
                                stg[:, :pnt, :],
                                table_d[cc][:],
                                idx_t[:, pt0 * 8 : (pt0 + pnt) * 8],
                                pnt * P,
                                pnt * P,
                                P,
                                single_packet=False,
                                queue_num=cc,
                            )
                            piece_tiles[key] = stg
                            # batched select build: sel[e, t', j] =
                            #   (iota[e, j] == dst[e, pt0+t']) * nrm[e, pt0+t']
                            sel = spool.tile([P, GT, P], dt.bfloat16, tag="sel")
                            nc.vector.tensor_tensor(
                                sel[:, :pnt, :],
                                iota_t[:].unsqueeze(1).broadcast_to([P, pnt, P]),
                                dst_t[:, pt0 : pt0 + pnt]
                                .unsqueeze(2)
                                .broadcast_to([P, pnt, P]),
                                op=alu.is_equal,
                            )
                            nc.vector.tensor_tensor(
                                sel[:, :pnt, :],
                                sel[:, :pnt, :],
                                nrm_t[:, pt0 : pt0 + pnt]
                                .unsqueeze(2)
                                .broadcast_to([P, pnt, P]),
                                op=alu.mult,
                            )
                            sel_tiles[key] = sel
                        stg = piece_tiles[key]
                        sel = sel_tiles[key]
                        nc.tensor.matmul(
                            pa[b][:],
                            stg[:, slot, :],
                            sel[:, slot, :],
                            start=False,
                            stop=(ch == last_ch[b] and t == int(T_run[ch][b]) - 1),
                        )
            for b in range(blo, bhi):
                nb = min(P, NPC - b * P)
                post(b, nb, pa[b])
            if after_group is not None:
                after_group(bhi)


# ---------------------------------------------------------------- entry


def build(z, edge_index, W_ih, W_hh, b_ih, b_hh, W1, b1, W2, b2, W3, b3):
    """Host prep + trace + compile. Returns (nc, in_maps)."""
    z = np.asarray(z, dtype=np.float32)
    W_ih = np.asarray(W_ih, dtype=np.float32)
    b = np.asarray(b_ih, dtype=np.float32) + np.asarray(b_hh, dtype=np.float32)

    arrs, sched = _prep_edges(edge_index)
    nc = _build_nc(sched)

    gi = {"i": 0, "g": 2, "o": 3}  # torch gate order i,f,g,o (f unused: c0=0)
    common = {
        "iota": np.ascontiguousarray(
            np.tile(np.arange(P, dtype=np.float32), (P, 1))
        ).astype(bf16),
        "ident": np.eye(P, dtype=np.float32).astype(bf16),
        "w1": np.asarray(W1, np.float32).astype(bf16),
        "w2": np.asarray(W2, np.float32).astype(bf16),
        "w3t": np.ascontiguousarray(np.asarray(W3, np.float32).T).astype(bf16),
        "b1": np.asarray(b1, np.float32).reshape(P, 1).copy(),
        "b2": np.asarray(b2, np.float32).reshape(P, 1).copy(),
        "b3": np.asarray(b3, np.float32).reshape(P, 1).copy(),
    }
    for g, k in gi.items():
        common[f"wih_{g}"] = np.ascontiguousarray(
            W_ih[k * P : (k + 1) * P, :].T
        ).astype(bf16)
        common[f"bg_{g}"] = b[k * P : (k + 1) * P].reshape(P, 1).copy()

    in_maps = []
    for c in range(NCORES):
        idx16, dstt, nrmt, d2t = arrs[c]
        m = dict(common)
        m["zT"] = np.ascontiguousarray(z[c * NPC : (c + 1) * NPC].T).astype(bf16)
        m["idx16"] = idx16
        m["dstv"] = dstt
        m["nrmv"] = nrmt
        m["dinv2"] = d2t
        in_maps.append(m)
    return nc, in_maps


def assemble(results):
    out = np.empty((N, P), np.float32)
    for c in range(NCORES):
        out[c * NPC : (c + 1) * NPC] = results[c]["outT"].T
    return out


def kernel(z, edge_index, W_ih, W_hh, b_ih, b_hh, W1, b1, W2, b2, W3, b3):
    nc, in_maps = build(z, edge_index, W_ih, W_hh, b_ih, b_hh, W1, b1, W2, b2, W3, b3)
    res = run_bass_kernel_spmd(nc, in_maps, core_ids=list(range(NCORES)))
    return assemble(res.results)


# revision 32
# speedup vs baseline: 12.9131x; 1.0266x over previous
"""Trainium2 Bass kernel for nn_Decoder (GNN message passing):
LSTM(1 step) -> GCNConv -> ReLU -> GCNConv -> Linear -> ReLU on a
100K-node / 1.6M-edge graph, SPMD across 8 NeuronCores.

Strategy (dst-node sharding):
- Core c owns nodes [c*12500, (c+1)*12500) and all edges into them.
- Per-node compute (LSTM, x@W transforms) runs feature-major [128, nodes]
  so all matmuls need zero transposes and biases are per-partition.
- The GCN propagate gathers transformed rows from a bf16 node-major table
  in DRAM (built via sub-AllGathers of the 8 shards, pipelined against the
  per-block transform matmuls) with gpsimd.dma_gather spread over 4 SWDGE
  queues (each queue runs on its own Q7 core pair), then scatter-adds via
  PE matmul with an on-chip selection matrix built in batched broadcast
  tensor_tensor ops (iota==dst -> * norm), accumulated in PSUM.
- Edge tiles are ordered group-major (groups of 8 dst blocks, chunk-major
  inside) so chunk-q gathers only wait on sub-AllGather q, and 8 PSUM
  accumulators carry a group across the 4 chunk sweeps.
"""

from contextlib import ExitStack

import numpy as np
import ml_dtypes

import concourse.bacc as bacc
import concourse.mybir as mybir
import concourse.tile as tile
from concourse.bass_utils import run_bass_kernel_spmd

P = 128
N = 100000
NCORES = 8
NPC = N // NCORES            # 12500 nodes per core
NBLK = (NPC + P - 1) // P    # 98 dst blocks per core (last has 84)
CH = 4                       # src chunks (int16 gather index limit)
QROWS = NPC // CH            # 3125: per-rank quarter contributed to a chunk
CHROWS = QROWS * NCORES      # 25000 rows per chunk table
GT = 24                      # max tiles (of 128 edges) per dma_gather
GRP = 6                      # dst blocks per PSUM accumulation group (6 PSUM
                             # banks for accumulators + 2 for the transform)
NGRP = (NBLK + GRP - 1) // GRP
MMB = 25                     # transform/bounce blocks per sub-AllGather piece
LSTM_CHUNK = 500             # nodes per LSTM/matmul column chunk

bf16 = ml_dtypes.bfloat16
f32 = np.float32


# ---------------------------------------------------------------- host prep


def _prep_edges(edge_index):
    """Sort/pad each core's incident edges into a cross-core-uniform tile
    schedule, ordered group-major: (dst-block group, src chunk, block, tile).
    Returns per-core device arrays + the static schedule."""
    src = np.asarray(edge_index[0], dtype=np.int64)
    dst = np.asarray(edge_index[1], dtype=np.int64)

    # self-loops enter deg/norm but are applied on-chip from the transform
    # stage (diag matmul), not gathered
    deg = np.bincount(dst, minlength=N).astype(np.float64) + 1.0
    dinv = 1.0 / np.sqrt(deg)
    norm = (dinv[src] * dinv[dst]).astype(np.float32)
    dinv2 = (dinv * dinv).astype(np.float32)

    core_of = dst // NPC
    per_core = []
    counts = np.zeros((NCORES, CH, NBLK), np.int64)
    for c in range(NCORES):
        m = core_of == c
        s = src[m]
        d = dst[m] - c * NPC
        w = norm[m]
        ch = (s % NPC) // QROWS
        o = np.lexsort((d, ch))
        s, d, w, ch = s[o], d[o], w[o], ch[o]
        b = d // P
        counts[c] = np.bincount(ch * NBLK + b, minlength=CH * NBLK).reshape(
            CH, NBLK
        )
        per_core.append((s, d, w, ch, b))

    # tiles per (chunk, block) run: padded to the max across cores
    T_run = (counts.max(axis=0) + P - 1) // P          # [CH, NBLK]

    # group-major tile order: for g: for ch: for b in group
    run_base = np.zeros((CH, NBLK), np.int64)
    t = 0
    seg = {}                                           # (g, ch) -> (t0, ntiles)
    for g in range(NGRP):
        blo, bhi = g * GRP, min(NBLK, (g + 1) * GRP)
        for ch in range(CH):
            t0 = t
            for b in range(blo, bhi):
                run_base[ch, b] = t
                t += int(T_run[ch, b])
            seg[(g, ch)] = (t0, t - t0)
    TT = int(t)
    NIDX = TT * P

    # gather pieces: per (g, ch) segment, consecutive groups of <= GT tiles
    pieces = {}
    tile_piece = {}                                    # gt -> (g, ch, pi, slot)
    for (g, ch), (t0, nt) in seg.items():
        plist = [(t0 + k, min(GT, nt - k)) for k in range(0, nt, GT)]
        pieces[(g, ch)] = plist
        for pi, (p0, pn) in enumerate(plist):
            for sl in range(pn):
                tile_piece[p0 + sl] = (g, ch, pi, sl)

    arrs = []
    for c in range(NCORES):
        s, d, w, ch, b = per_core[c]
        gid = ch * NBLK + b
        cnt = counts[c].reshape(-1)
        gstart = np.concatenate([[0], np.cumsum(cnt)[:-1]])
        within = np.arange(len(s)) - gstart[gid]
        pos = run_base[ch, b] * P + within

        idxs = np.zeros(NIDX, np.int16)                 # pad -> row 0 (valid)
        # chunk q table = concat over ranks of each rank's q-th quarter
        idxs[pos] = ((s // NPC) * QROWS + (s % QROWS)).astype(np.int16)
        dstv = np.full(NIDX, -1.0, np.float32)          # pad -> no dst match
        dstv[pos] = (d - b * P).astype(np.float32)
        nrmv = np.zeros(NIDX, np.float32)
        nrmv[pos] = w

        idx16 = np.tile(np.ascontiguousarray(idxs.reshape(-1, 16).T), (8, 1))
        dstt = np.ascontiguousarray(dstv.reshape(TT, P).T).astype(bf16)
        nrmt = np.ascontiguousarray(nrmv.reshape(TT, P).T).astype(bf16)
        d2 = np.zeros((NBLK, P), np.float32)          # [block, node-in-block]
        d2.reshape(-1)[:NPC] = dinv2[c * NPC : (c + 1) * NPC]
        d2t = np.ascontiguousarray(d2.T).astype(bf16)  # [P, NBLK]
        arrs.append((idx16, dstt, nrmt, d2t))

    sched = dict(
        T_run=T_run, run_base=run_base, TT=TT, NIDX=NIDX,
        pieces=pieces, tile_piece=tile_piece,
    )
    return arrs, sched


# ---------------------------------------------------------------- device


def _build_nc(sched):
    TT = sched["TT"]
    NIDX = sched["NIDX"]
    dt = mybir.dt
    alu = mybir.AluOpType
    act = mybir.ActivationFunctionType

    nc = bacc.Bacc(
        "TRN2",
        target_bir_lowering=False,
        debug=False,
        num_devices=NCORES,
        num_swdge_queues=4,
    )

    # ---- I/O
    zT_d = nc.dram_tensor("zT", [P, NPC], dt.bfloat16, kind="ExternalInput")
    idx_d = nc.dram_tensor("idx16", [P, NIDX // 16], dt.int16, kind="ExternalInput")
    dst_d = nc.dram_tensor("dstv", [P, TT], dt.bfloat16, kind="ExternalInput")
    nrm_d = nc.dram_tensor("nrmv", [P, TT], dt.bfloat16, kind="ExternalInput")
    iota_d = nc.dram_tensor("iota", [P, P], dt.bfloat16, kind="ExternalInput")
    ident_d = nc.dram_tensor("ident", [P, P], dt.bfloat16, kind="ExternalInput")
    dinv2_d = nc.dram_tensor("dinv2", [P, NBLK], dt.bfloat16, kind="ExternalInput")
    wih_d = {
        g: nc.dram_tensor(f"wih_{g}", [P, P], dt.bfloat16, kind="ExternalInput")
        for g in "igo"
    }
    bg_d = {
        g: nc.dram_tensor(f"bg_{g}", [P, 1], dt.float32, kind="ExternalInput")
        for g in "igo"
    }
    w1_d = nc.dram_tensor("w1", [P, P], dt.bfloat16, kind="ExternalInput")
    w2_d = nc.dram_tensor("w2", [P, P], dt.bfloat16, kind="ExternalInput")
    w3t_d = nc.dram_tensor("w3t", [P, P], dt.bfloat16, kind="ExternalInput")
    b1_d = nc.dram_tensor("b1", [P, 1], dt.float32, kind="ExternalInput")
    b2_d = nc.dram_tensor("b2", [P, 1], dt.float32, kind="ExternalInput")
    b3_d = nc.dram_tensor("b3", [P, 1], dt.float32, kind="ExternalInput")
    out_d = nc.dram_tensor("outT", [P, NPC], dt.float32, kind="ExternalOutput")

    bounce = [nc.dram_tensor(f"bounce{l}", [NPC, P], dt.bfloat16) for l in range(2)]
    table = [
        [
            nc.dram_tensor(
                f"table{l}_{q}", [CHROWS, P], dt.bfloat16, addr_space="Shared"
            )
            for q in range(CH)
        ]
        for l in range(2)
    ]

    with tile.TileContext(nc) as tc, ExitStack() as ctx:
        konst = ctx.enter_context(tc.tile_pool(name="konst", bufs=1))
        big = ctx.enter_context(tc.tile_pool(name="big", bufs=1))

        def load_const(handle, shape, dtype):
            t = konst.tile(shape, dtype, tag=handle.name)
            nc.sync.dma_start(t[:], handle[:])
            return t

        iota_t = load_const(iota_d, [P, P], dt.bfloat16)
        ident_t = load_const(ident_d, [P, P], dt.bfloat16)
        dinv2_t = load_const(dinv2_d, [P, NBLK], dt.bfloat16)
        wih_t = {g: load_const(wih_d[g], [P, P], dt.bfloat16) for g in "igo"}
        bg_t = {g: load_const(bg_d[g], [P, 1], dt.float32) for g in "igo"}
        w1_t = load_const(w1_d, [P, P], dt.bfloat16)
        w2_t = load_const(w2_d, [P, P], dt.bfloat16)
        w3t_t = load_const(w3t_d, [P, P], dt.bfloat16)
        b1_t = load_const(b1_d, [P, 1], dt.float32)
        b2_t = load_const(b2_d, [P, 1], dt.float32)
        b3_t = load_const(b3_d, [P, 1], dt.float32)
        idx_t = load_const(idx_d, [P, NIDX // 16], dt.int16)
        dst_t = load_const(dst_d, [P, TT], dt.bfloat16)
        nrm_t = load_const(nrm_d, [P, TT], dt.bfloat16)

        xT_t = big.tile([P, NPC], dt.bfloat16, tag="xT")  # x1T then x2T

        mm_ps = ctx.enter_context(tc.tile_pool(name="m_ps", bufs=2, space="PSUM"))
        mm_sb = ctx.enter_context(tc.tile_pool(name="m_sb", bufs=1))

        # ---------------- phase 1: LSTM -> hT (feature-major, bf16)
        with tc.tile_pool(name="h_pool", bufs=1) as hpool:
            hT_t = hpool.tile([P, NPC], dt.bfloat16, tag="hT")
            with (
                tc.tile_pool(name="lstm_sb", bufs=1) as lsb,
                tc.tile_pool(name="lstm_ps", bufs=6, space="PSUM") as lps,
                tc.tile_pool(name="lstm_tr", bufs=8) as ltr,
            ):
                zT_t = lsb.tile([P, NPC], dt.bfloat16, tag="zT")
                nc.sync.dma_start(zT_t[:], zT_d[:])

                pipe1 = _MMPipe(
                    nc, tc, mm_ps, mm_sb, hT_t, w1_t, bounce[0], table[0]
                )
                nchunk = (NPC + LSTM_CHUNK - 1) // LSTM_CHUNK
                for k in range(nchunk):
                    c0 = k * LSTM_CHUNK
                    c1 = min(NPC, c0 + LSTM_CHUNK)
                    w = c1 - c0
                    gate = {}
                    for g in "igo":
                        ps = lps.tile([P, LSTM_CHUNK], dt.float32, tag="ps")
                        nc.tensor.matmul(
                            ps[:, :w], wih_t[g][:], zT_t[:, c0:c1], start=True, stop=True
                        )
                        fn = act.Tanh if g == "g" else act.Sigmoid
                        sg = ltr.tile([P, LSTM_CHUNK], dt.bfloat16, tag="sg" + g)
                        nc.scalar.activation(sg[:, :w], ps[:, :w], fn, bias=bg_t[g][:])
                        gate[g] = sg
                    ct = ltr.tile([P, LSTM_CHUNK], dt.bfloat16, tag="ct")
                    nc.vector.tensor_tensor(
                        ct[:, :w], gate["i"][:, :w], gate["g"][:, :w], op=alu.mult
                    )
                    th = ltr.tile([P, LSTM_CHUNK], dt.bfloat16, tag="th")
                    nc.scalar.activation(th[:, :w], ct[:, :w], act.Tanh)
                    nc.vector.tensor_tensor(
                        hT_t[:, c0:c1], gate["o"][:, :w], th[:, :w], op=alu.mult
                    )
                    # phase 2 interleaved: transform blocks fully covered by
                    # the LSTM so sub-AllGathers start during the LSTM sweep
                    pipe1.advance(c1 // P)

            pipe1.advance(NBLK)

        with (
            tc.tile_pool(name="stag", bufs=7) as stag,
            tc.tile_pool(name="spool", bufs=5) as spool,
        ):
            # ------------- phase 3: edge layer 1 -> x1T = relu(agg + b1),
            # interleaved with phase 4 (m2 = x1 @ W2 -> bounce1 + AGs)
            def post1(b, nb, pa):
                nc.scalar.activation(
                    xT_t[:, b * P : b * P + nb], pa[:, :nb], act.Relu, bias=b1_t[:]
                )

            pipe2 = _MMPipe(nc, tc, mm_ps, mm_sb, xT_t, w2_t, bounce[1], table[1])
            _edge_phase(
                nc, tc, table[0], sched, idx_t, dst_t, nrm_t, iota_t,
                stag, spool, post1, pipe1.stage, dinv2_t, ident_t,
                after_group=pipe2.advance,
            )
            pipe2.advance(NBLK)

            # ------------- phase 5: edge layer 2 -> x2T = agg + b2 (no relu)
            def post2(b, nb, pa):
                nc.vector.tensor_scalar(
                    xT_t[:, b * P : b * P + nb], pa[:, :nb], b2_t[:], None, op0=alu.add
                )

            _edge_phase(
                nc, tc, table[1], sched, idx_t, dst_t, nrm_t, iota_t,
                stag, spool, post2, pipe2.stage, dinv2_t, ident_t,
            )

        # ---------------- phase 6: outT = relu(W3T.T @ x2T + b3)
        with (
            tc.tile_pool(name="out_ps", bufs=3, space="PSUM") as ops,
            tc.tile_pool(name="out_sb", bufs=3) as osb,
        ):
            nchunk = (NPC + LSTM_CHUNK - 1) // LSTM_CHUNK
            for k in range(nchunk):
                c0 = k * LSTM_CHUNK
                c1 = min(NPC, c0 + LSTM_CHUNK)
                w = c1 - c0
                ps = ops.tile([P, LSTM_CHUNK], dt.float32, tag="ps")
                nc.tensor.matmul(
                    ps[:, :w], w3t_t[:], xT_t[:, c0:c1], start=True, stop=True
                )
                ot = osb.tile([P, LSTM_CHUNK], dt.float32, tag="ot")
                nc.scalar.activation(ot[:, :w], ps[:, :w], act.Relu, bias=b3_t[:])
                nc.sync.dma_start(out_d[:, c0:c1], ot[:, :w])

    nc.compile()
    return nc


class _MMPipe:
    """Per-block transform (featT block @ W -> node-major bf16 stage),
    with bounce-DMA + sub-AllGather emitted per MMB-block piece so the
    collectives overlap trailing compute."""

    def __init__(self, nc, tc, mm_ps, mm_sb, featT, w_t, bounce_d, tables_d):
        self.nc = nc
        self.mm_ps = mm_ps
        self.mm_sb = mm_sb
        self.featT = featT
        self.w_t = w_t
        self.bounce_d = bounce_d
        self.tables_d = tables_d
        # per-pipe stage: edge phase l reads pipe l's stage for the self-loop
        # diag matmuls, so the two layers' stages must coexist
        self.stage = mm_sb.tile(
            [P, NBLK * P], mybir.dt.bfloat16,
            tag=f"mst_{bounce_d.name}", name=f"stage_{bounce_d.name}",
        )
        self.done_b = 0
        self.piece = 0

    def advance(self, bend):
        nc = self.nc
        dt = mybir.dt
        act = mybir.ActivationFunctionType
        for b in range(self.done_b, bend):
            nb = min(P, NPC - b * P)
            pm = self.mm_ps.tile([P, P], dt.float32, tag="pm")
            nc.tensor.matmul(
                pm[:nb, :],
                self.featT[:, b * P : b * P + nb],
                self.w_t[:],
                start=True,
                stop=True,
            )
            nc.scalar.activation(
                self.stage[:nb, b * P : (b + 1) * P], pm[:nb, :], act.Copy
            )
        self.done_b = bend

        while self.piece < CH and (
            (self.piece + 1) * MMB <= bend or bend == NBLK
        ):
            p = self.piece
            blo = p * MMB
            bhi = min(NBLK, blo + MMB)
            full = min(bhi * P, (NPC // P) * P)
            nc.sync.dma_start(
                self.bounce_d[blo * P : full, :].rearrange(
                    "(b p) f -> p b f", p=P
                ),
                self.stage[:, blo * P : full].rearrange("p (b f) -> p b f", f=P),
            )
            if bhi * P > full:          # tail remainder rows (12416..12500)
                rem = NPC - full
                nc.sync.dma_start(
                    self.bounce_d[full:, :], self.stage[:rem, full:]
                )
            nc.gpsimd.collective_compute(
                "AllGather",
                mybir.AluOpType.bypass,
                replica_groups=[list(range(NCORES))],
                ins=[self.bounce_d[p * QROWS : (p + 1) * QROWS, :]],
                outs=[self.tables_d[p][:]],
            )
            self.piece += 1


def _edge_phase(
    nc, tc, table_d, sched, idx_t, dst_t, nrm_t, iota_t, stag, spool, post,
    stage, dinv2_t, ident_t, after_group=None,
):
    dt = mybir.dt
    alu = mybir.AluOpType
    T_run = sched["T_run"]
    run_base = sched["run_base"]
    pieces = sched["pieces"]
    tile_piece = sched["tile_piece"]

    last_ch = {}
    for b in range(NBLK):
        nz = [ch for ch in range(CH) if T_run[ch][b] > 0]
        last_ch[b] = nz[-1] if nz else -1

    piece_tiles = {}
    sel_tiles = {}
    with tc.tile_pool(name="agg_ps", bufs=1, space="PSUM") as aps:
        for g in range(NGRP):
            blo, bhi = g * GRP, min(NBLK, (g + 1) * GRP)
            ng = bhi - blo
            pa = {}
            for b in range(blo, bhi):
                pa_b = aps.tile(
                    [P, P], dt.float32, tag=f"pa{b - blo}", name=f"pa_{g}_{b}"
                )
                pa[b] = pa_b
            # self-loops: pa[b] starts as stage[b]^T * dinv2 (diag select)
            sdg = spool.tile(
                [P, GRP, P], dt.bfloat16, tag="sdg", bufs=2, name="sdg"
            )
            nc.vector.tensor_tensor(
                sdg[:, :ng, :],
                ident_t[:].unsqueeze(1).broadcast_to([P, ng, P]),
                dinv2_t[:, blo:bhi].unsqueeze(2).broadcast_to([P, ng, P]),
                op=alu.mult,
            )
            for b in range(blo, bhi):
                nb = min(P, NPC - b * P)
                nc.tensor.matmul(
                    pa[b][:],
                    stage[:nb, b * P : (b + 1) * P],
                    sdg[:nb, b - blo, :],
                    start=True,
                    stop=(last_ch[b] < 0),
                )
            for ch in range(CH):
                for b in range(blo, bhi):
                    for t in range(int(T_run[ch][b])):
                        gt = int(run_base[ch][b]) + t
                        gg, cc, pi, slot = tile_piece[gt]
                        key = (gg, cc, pi)
                        if key not in piece_tiles:
                            pt0, pnt = pieces[(gg, cc)][pi]
                            stg = stag.tile([P, GT, P], dt.bfloat16, tag="stag")
                            nc.gpsimd.dma_gather(
                                stg[:, :pnt, :],
                                table_d[cc][:],
                                idx_t[:, pt0 * 8 : (pt0 + pnt) * 8],
                                pnt * P,
                                pnt * P,
                                P,
                                single_packet=False,
                                queue_num=cc,
                            )
                            piece_tiles[key] = stg
                            # batched select build: sel[e, t', j] =
                            #   (iota[e, j] == dst[e, pt0+t']) * nrm[e, pt0+t']
                            sel = spool.tile([P, GT, P], dt.bfloat16, tag="sel")
                            nc.vector.tensor_tensor(
                                sel[:, :pnt, :],
                                iota_t[:].unsqueeze(1).broadcast_to([P, pnt, P]),
                                dst_t[:, pt0 : pt0 + pnt]
                                .unsqueeze(2)
                                .broadcast_to([P, pnt, P]),
                                op=alu.is_equal,
                            )
                            nc.vector.tensor_tensor(
                                sel[:, :pnt, :],
                                sel[:, :pnt, :],
                                nrm_t[:, pt0 : pt0 + pnt]
                                .unsqueeze(2)
                                .broadcast_to([P, pnt, P]),
                                op=alu.mult,
                            )
                            sel_tiles[key] = sel
                        stg = piece_tiles[key]
                        sel = sel_tiles[key]
                        nc.tensor.matmul(
                            pa[b][:],
                            stg[:, slot, :],
                            sel[:, slot, :],
                            start=False,
                            stop=(ch == last_ch[b] and t == int(T_run[ch][b]) - 1),
                        )
            for b in range(blo, bhi):
                nb = min(P, NPC - b * P)
                post(b, nb, pa[b])
            if after_group is not None:
                after_group(bhi)


# ---------------------------------------------------------------- entry


def build(z, edge_index, W_ih, W_hh, b_ih, b_hh, W1, b1, W2, b2, W3, b3):
    """Host prep + trace + compile. Returns (nc, in_maps)."""
    z = np.asarray(z, dtype=np.float32)
    W_ih = np.asarray(W_ih, dtype=np.float32)
    b = np.asarray(b_ih, dtype=np.float32) + np.asarray(b_hh, dtype=np.float32)

    arrs, sched = _prep_edges(edge_index)
    nc = _build_nc(sched)

    gi = {"i": 0, "g": 2, "o": 3}  # torch gate order i,f,g,o (f unused: c0=0)
    common = {
        "iota": np.ascontiguousarray(
            np.tile(np.arange(P, dtype=np.float32), (P, 1))
        ).astype(bf16),
        "ident": np.eye(P, dtype=np.float32).astype(bf16),
        "w1": np.asarray(W1, np.float32).astype(bf16),
        "w2": np.asarray(W2, np.float32).astype(bf16),
        "w3t": np.ascontiguousarray(np.asarray(W3, np.float32).T).astype(bf16),
        "b1": np.asarray(b1, np.float32).reshape(P, 1).copy(),
        "b2": np.asarray(b2, np.float32).reshape(P, 1).copy(),
        "b3": np.asarray(b3, np.float32).reshape(P, 1).copy(),
    }
    for g, k in gi.items():
        common[f"wih_{g}"] = np.ascontiguousarray(
            W_ih[k * P : (k + 1) * P, :].T
        ).astype(bf16)
        common[f"bg_{g}"] = b[k * P : (k + 1) * P].reshape(P, 1).copy()

    in_maps = []
    for c in range(NCORES):
        idx16, dstt, nrmt, d2t = arrs[c]
        m = dict(common)
        m["zT"] = np.ascontiguousarray(z[c * NPC : (c + 1) * NPC].T).astype(bf16)
        m["idx16"] = idx16
        m["dstv"] = dstt
        m["nrmv"] = nrmt
        m["dinv2"] = d2t
        in_maps.append(m)
    return nc, in_maps


def assemble(results):
    out = np.empty((N, P), np.float32)
    for c in range(NCORES):
        out[c * NPC : (c + 1) * NPC] = results[c]["outT"].T
    return out


def kernel(z, edge_index, W_ih, W_hh, b_ih, b_hh, W1, b1, W2, b2, W3, b3):
    nc, in_maps = build(z, edge_index, W_ih, W_hh, b_ih, b_hh, W1, b1, W2, b2, W3, b3)
    res = run_bass_kernel_spmd(nc, in_maps, core_ids=list(range(NCORES)))
    return assemble(res.results)


# revision 39
# speedup vs baseline: 17.6339x; 1.3656x over previous
"""Trainium2 Bass kernel for nn_Decoder (GNN message passing):
LSTM(1 step) -> GCNConv -> ReLU -> GCNConv -> Linear -> ReLU on a
100K-node / 1.6M-edge graph, SPMD across 8 NeuronCores.

Strategy (dst-node sharding):
- Core c owns nodes [c*12500, (c+1)*12500) and all edges into them.
- Per-node compute (LSTM, x@W transforms) runs feature-major [128, nodes]
  so all matmuls need zero transposes and biases are per-partition.
- The GCN propagate gathers transformed rows from a bf16 node-major table
  in DRAM (built via sub-AllGathers of the 8 shards, pipelined against the
  per-block transform matmuls) with gpsimd.dma_gather spread over 4 SWDGE
  queues (each queue runs on its own Q7 core pair), then scatter-adds via
  PE matmul with an on-chip selection matrix built in batched broadcast
  tensor_tensor ops (iota==dst -> * norm), accumulated in PSUM.
- Edge tiles are ordered group-major (groups of 8 dst blocks, chunk-major
  inside) so chunk-q gathers only wait on sub-AllGather q, and 8 PSUM
  accumulators carry a group across the 4 chunk sweeps.
"""

from contextlib import ExitStack

import numpy as np
import ml_dtypes

import concourse.bacc as bacc
import concourse.mybir as mybir
import concourse.tile as tile
from concourse.bass_utils import run_bass_kernel_spmd

P = 128
N = 100000
NCORES = 8
NPC = N // NCORES            # 12500 nodes per core
NBLK = (NPC + P - 1) // P    # 98 dst blocks per core (last has 84)
CH = 4                       # src chunks (int16 gather index limit)
QROWS = NPC // CH            # 3125: per-rank quarter contributed to a chunk
CHROWS = QROWS * NCORES      # 25000 rows per chunk table
GT = 24                      # max tiles (of 128 edges) per dma_gather
GRP = 6                      # dst blocks per PSUM accumulation group (6 PSUM
                             # banks for accumulators + 2 for the transform)
NGRP = (NBLK + GRP - 1) // GRP
MMB = 25                     # transform/bounce blocks per sub-AllGather piece
LSTM_CHUNK = 500             # nodes per LSTM/matmul column chunk

bf16 = ml_dtypes.bfloat16
f32 = np.float32


# ---------------------------------------------------------------- host prep


def _prep_edges(edge_index):
    """Sort/pad each core's incident edges into a cross-core-uniform tile
    schedule. Edges are packed densely per (dst-block group, src chunk)
    SEGMENT (tiles may cross dst-block boundaries inside a segment); the
    device runs one matmul per (tile, touched block) with per-matmul select
    columns precomputed here. Returns per-core arrays + the schedule."""
    src = np.asarray(edge_index[0], dtype=np.int64)
    dst = np.asarray(edge_index[1], dtype=np.int64)

    # self-loops enter deg/norm but are applied on-chip from the transform
    # stage (diag matmul), not gathered
    deg = np.bincount(dst, minlength=N).astype(np.float64) + 1.0
    dinv = 1.0 / np.sqrt(deg)
    norm = (dinv[src] * dinv[dst]).astype(np.float32)
    dinv2 = (dinv * dinv).astype(np.float32)

    NSEG = NGRP * CH

    core_of = dst // NPC
    per_core = []
    seg_cnt = np.zeros((NCORES, NSEG), np.int64)
    for c in range(NCORES):
        m = core_of == c
        s = src[m]
        d = dst[m] - c * NPC
        w = norm[m]
        ch = (s % NPC) // QROWS
        o = np.lexsort((d, ch))
        s, d, w, ch = s[o], d[o], w[o], ch[o]
        b = d // P
        sid = (b // GRP) * CH + ch
        seg_cnt[c] = np.bincount(sid, minlength=NSEG)
        per_core.append((s, d, w, ch, b, sid))

    # tiles per segment: dense packing, padded to the max across cores
    seg_tiles = (seg_cnt.max(axis=0) + P - 1) // P      # [NSEG]
    seg_base = np.concatenate([[0], np.cumsum(seg_tiles)[:-1]])
    TT = int(seg_tiles.sum())
    NIDX = TT * P

    # per-core slot position of each edge + (seg, tile-in-seg, block) triples
    core_pos = []
    touched = [set() for _ in range(NSEG)]              # (tloc, b) per segment
    for c in range(NCORES):
        s, d, w, ch, b, sid = per_core[c]
        # edges are sorted (ch, d) -> segments appear in ch-major order
        skey = ch * NGRP + (b // GRP)
        cnt_k = np.bincount(skey, minlength=NSEG)
        kstart = np.concatenate([[0], np.cumsum(cnt_k)[:-1]])
        within = np.arange(len(s)) - kstart[skey]
        pos = seg_base[sid] * P + within
        tloc = within // P
        core_pos.append((pos, tloc))
        for ss in range(NSEG):
            msk = sid == ss
            for tb in set(zip(tloc[msk].tolist(), b[msk].tolist())):
                touched[ss].add(tb)

    # matmul schedule: emission order (g, ch, tile, block); pieces of <= GT
    # tiles per gather call with their matmul lists
    mindex = {}                                        # (sid, tloc, b) -> m
    pieces = {}                                        # (g, ch) -> [piece...]
    last_m_of_b = np.full(NBLK, -1, np.int64)
    MT = 0
    GTM = 0
    for g in range(NGRP):
        for ch in range(CH):
            sid = g * CH + ch
            nt = int(seg_tiles[sid])
            t0 = int(seg_base[sid])
            tb_sorted = sorted(touched[sid])
            plist = []
            for k in range(0, nt, GT):
                pn = min(GT, nt - k)
                mlist = []
                m0 = MT
                for tloc, b in tb_sorted:
                    if k <= tloc < k + pn:
                        mindex[(sid, tloc, b)] = MT
                        mlist.append((MT - m0, tloc - k, b))
                        last_m_of_b[b] = MT
                        MT += 1
                plist.append((t0 + k, pn, m0, mlist))
                GTM = max(GTM, len(mlist))
            pieces[(g, ch)] = plist

    arrs = []
    for c in range(NCORES):
        s, d, w, ch, b, sid = per_core[c]
        pos, tloc = core_pos[c]

        idxs = np.zeros(NIDX, np.int16)                 # pad -> row 0 (valid)
        # chunk q table = concat over ranks of each rank's q-th quarter
        idxs[pos] = ((s // NPC) * QROWS + (s % QROWS)).astype(np.int16)

        marr = np.fromiter(
            (mindex[(int(ss), int(tt), int(bb))] for ss, tt, bb in zip(sid, tloc, b)),
            dtype=np.int64,
            count=len(s),
        )
        row = pos % P
        dstv = np.full(MT * P, -1.0, np.float32)        # default: no dst match
        dstv[marr * P + row] = (d - b * P).astype(np.float32)
        nrmv = np.zeros(MT * P, np.float32)
        nrmv[marr * P + row] = w

        idx16 = np.tile(np.ascontiguousarray(idxs.reshape(-1, 16).T), (8, 1))
        dstt = np.ascontiguousarray(dstv.reshape(MT, P).T).astype(bf16)
        nrmt = np.ascontiguousarray(nrmv.reshape(MT, P).T).astype(bf16)
        d2 = np.zeros((NBLK, P), np.float32)          # [block, node-in-block]
        d2.reshape(-1)[:NPC] = dinv2[c * NPC : (c + 1) * NPC]
        d2t = np.ascontiguousarray(d2.T).astype(bf16)  # [P, NBLK]
        arrs.append((idx16, dstt, nrmt, d2t))

    sched = dict(
        TT=TT, NIDX=NIDX, MT=MT, GTM=GTM,
        pieces=pieces, last_m_of_b=last_m_of_b,
    )
    return arrs, sched


# ---------------------------------------------------------------- device


def _build_nc(sched):
    NIDX = sched["NIDX"]
    MT = sched["MT"]
    dt = mybir.dt
    alu = mybir.AluOpType
    act = mybir.ActivationFunctionType

    nc = bacc.Bacc(
        "TRN2",
        target_bir_lowering=False,
        debug=False,
        num_devices=NCORES,
        num_swdge_queues=4,
    )

    # ---- I/O
    zT_d = nc.dram_tensor("zT", [P, NPC], dt.bfloat16, kind="ExternalInput")
    idx_d = nc.dram_tensor("idx16", [P, NIDX // 16], dt.int16, kind="ExternalInput")
    dst_d = nc.dram_tensor("dstv", [P, MT], dt.bfloat16, kind="ExternalInput")
    nrm_d = nc.dram_tensor("nrmv", [P, MT], dt.bfloat16, kind="ExternalInput")
    iota_d = nc.dram_tensor("iota", [P, P], dt.bfloat16, kind="ExternalInput")
    ident_d = nc.dram_tensor("ident", [P, P], dt.bfloat16, kind="ExternalInput")
    dinv2_d = nc.dram_tensor("dinv2", [P, NBLK], dt.bfloat16, kind="ExternalInput")
    wih_d = {
        g: nc.dram_tensor(f"wih_{g}", [P, P], dt.bfloat16, kind="ExternalInput")
        for g in "igo"
    }
    bg_d = {
        g: nc.dram_tensor(f"bg_{g}", [P, 1], dt.float32, kind="ExternalInput")
        for g in "igo"
    }
    w1_d = nc.dram_tensor("w1", [P, P], dt.bfloat16, kind="ExternalInput")
    w2_d = nc.dram_tensor("w2", [P, P], dt.bfloat16, kind="ExternalInput")
    w3t_d = nc.dram_tensor("w3t", [P, P], dt.bfloat16, kind="ExternalInput")
    b1_d = nc.dram_tensor("b1", [P, 1], dt.float32, kind="ExternalInput")
    b2_d = nc.dram_tensor("b2", [P, 1], dt.float32, kind="ExternalInput")
    b3_d = nc.dram_tensor("b3", [P, 1], dt.float32, kind="ExternalInput")
    out_d = nc.dram_tensor("outT", [P, NPC], dt.float32, kind="ExternalOutput")

    bounce = [nc.dram_tensor(f"bounce{l}", [NPC, P], dt.bfloat16) for l in range(2)]
    table = [
        [
            nc.dram_tensor(
                f"table{l}_{q}", [CHROWS, P], dt.bfloat16, addr_space="Shared"
            )
            for q in range(CH)
        ]
        for l in range(2)
    ]

    with tile.TileContext(nc) as tc, ExitStack() as ctx:
        konst = ctx.enter_context(tc.tile_pool(name="konst", bufs=1))
        big = ctx.enter_context(tc.tile_pool(name="big", bufs=1))

        def load_const(handle, shape, dtype):
            t = konst.tile(shape, dtype, tag=handle.name)
            nc.sync.dma_start(t[:], handle[:])
            return t

        iota_t = load_const(iota_d, [P, P], dt.bfloat16)
        ident_t = load_const(ident_d, [P, P], dt.bfloat16)
        dinv2_t = load_const(dinv2_d, [P, NBLK], dt.bfloat16)
        wih_t = {g: load_const(wih_d[g], [P, P], dt.bfloat16) for g in "igo"}
        bg_t = {g: load_const(bg_d[g], [P, 1], dt.float32) for g in "igo"}
        w1_t = load_const(w1_d, [P, P], dt.bfloat16)
        w2_t = load_const(w2_d, [P, P], dt.bfloat16)
        w3t_t = load_const(w3t_d, [P, P], dt.bfloat16)
        b1_t = load_const(b1_d, [P, 1], dt.float32)
        b2_t = load_const(b2_d, [P, 1], dt.float32)
        b3_t = load_const(b3_d, [P, 1], dt.float32)
        idx_t = load_const(idx_d, [P, NIDX // 16], dt.int16)
        dst_t = load_const(dst_d, [P, MT], dt.bfloat16)
        nrm_t = load_const(nrm_d, [P, MT], dt.bfloat16)

        xT_t = big.tile([P, NPC], dt.bfloat16, tag="xT")  # x1T then x2T

        mm_ps = ctx.enter_context(tc.tile_pool(name="m_ps", bufs=2, space="PSUM"))
        mm_sb = ctx.enter_context(tc.tile_pool(name="m_sb", bufs=1))

        # ---------------- phase 1: LSTM -> hT (feature-major, bf16)
        with tc.tile_pool(name="h_pool", bufs=1) as hpool:
            hT_t = hpool.tile([P, NPC], dt.bfloat16, tag="hT")
            with (
                tc.tile_pool(name="lstm_sb", bufs=1) as lsb,
                tc.tile_pool(name="lstm_ps", bufs=6, space="PSUM") as lps,
                tc.tile_pool(name="lstm_tr", bufs=8) as ltr,
            ):
                zT_t = lsb.tile([P, NPC], dt.bfloat16, tag="zT")
                nc.sync.dma_start(zT_t[:], zT_d[:])

                pipe1 = _MMPipe(
                    nc, tc, mm_ps, mm_sb, hT_t, w1_t, bounce[0], table[0]
                )
                nchunk = (NPC + LSTM_CHUNK - 1) // LSTM_CHUNK
                for k in range(nchunk):
                    c0 = k * LSTM_CHUNK
                    c1 = min(NPC, c0 + LSTM_CHUNK)
                    w = c1 - c0
                    gate = {}
                    for g in "igo":
                        ps = lps.tile([P, LSTM_CHUNK], dt.float32, tag="ps")
                        nc.tensor.matmul(
                            ps[:, :w], wih_t[g][:], zT_t[:, c0:c1], start=True, stop=True
                        )
                        fn = act.Tanh if g == "g" else act.Sigmoid
                        sg = ltr.tile([P, LSTM_CHUNK], dt.bfloat16, tag="sg" + g)
                        nc.scalar.activation(sg[:, :w], ps[:, :w], fn, bias=bg_t[g][:])
                        gate[g] = sg
                    ct = ltr.tile([P, LSTM_CHUNK], dt.bfloat16, tag="ct")
                    nc.vector.tensor_tensor(
                        ct[:, :w], gate["i"][:, :w], gate["g"][:, :w], op=alu.mult
                    )
                    th = ltr.tile([P, LSTM_CHUNK], dt.bfloat16, tag="th")
                    nc.scalar.activation(th[:, :w], ct[:, :w], act.Tanh)
                    nc.vector.tensor_tensor(
                        hT_t[:, c0:c1], gate["o"][:, :w], th[:, :w], op=alu.mult
                    )
                    # phase 2 interleaved: transform blocks fully covered by
                    # the LSTM so sub-AllGathers start during the LSTM sweep
                    pipe1.advance(c1 // P)

            pipe1.advance(NBLK)

        with (
            tc.tile_pool(name="stag", bufs=7) as stag,
            tc.tile_pool(name="spool", bufs=4) as spool,
        ):
            # ------------- phase 3: edge layer 1 -> x1T = relu(agg + b1),
            # interleaved with phase 4 (m2 = x1 @ W2 -> bounce1 + AGs)
            def post1(b, nb, pa):
                nc.scalar.activation(
                    xT_t[:, b * P : b * P + nb], pa[:, :nb], act.Relu, bias=b1_t[:]
                )

            pipe2 = _MMPipe(nc, tc, mm_ps, mm_sb, xT_t, w2_t, bounce[1], table[1])
            _edge_phase(
                nc, tc, table[0], sched, idx_t, dst_t, nrm_t, iota_t,
                stag, spool, post1, pipe1.stage, dinv2_t, ident_t,
                after_group=pipe2.advance,
            )
            pipe2.advance(NBLK)

            # ------------- phase 5: edge layer 2 -> x2T = agg + b2 (no relu)
            def post2(b, nb, pa):
                nc.vector.tensor_scalar(
                    xT_t[:, b * P : b * P + nb], pa[:, :nb], b2_t[:], None, op0=alu.add
                )

            _edge_phase(
                nc, tc, table[1], sched, idx_t, dst_t, nrm_t, iota_t,
                stag, spool, post2, pipe2.stage, dinv2_t, ident_t,
            )

        # ---------------- phase 6: outT = relu(W3T.T @ x2T + b3)
        with (
            tc.tile_pool(name="out_ps", bufs=3, space="PSUM") as ops,
            tc.tile_pool(name="out_sb", bufs=3) as osb,
        ):
            nchunk = (NPC + LSTM_CHUNK - 1) // LSTM_CHUNK
            for k in range(nchunk):
                c0 = k * LSTM_CHUNK
                c1 = min(NPC, c0 + LSTM_CHUNK)
                w = c1 - c0
                ps = ops.tile([P, LSTM_CHUNK], dt.float32, tag="ps")
                nc.tensor.matmul(
                    ps[:, :w], w3t_t[:], xT_t[:, c0:c1], start=True, stop=True
                )
                ot = osb.tile([P, LSTM_CHUNK], dt.float32, tag="ot")
                nc.scalar.activation(ot[:, :w], ps[:, :w], act.Relu, bias=b3_t[:])
                nc.sync.dma_start(out_d[:, c0:c1], ot[:, :w])

    nc.compile()
    return nc


class _MMPipe:
    """Per-block transform (featT block @ W -> node-major bf16 stage),
    with bounce-DMA + sub-AllGather emitted per MMB-block piece so the
    collectives overlap trailing compute."""

    def __init__(self, nc, tc, mm_ps, mm_sb, featT, w_t, bounce_d, tables_d):
        self.nc = nc
        self.mm_ps = mm_ps
        self.mm_sb = mm_sb
        self.featT = featT
        self.w_t = w_t
        self.bounce_d = bounce_d
        self.tables_d = tables_d
        # per-pipe stage: edge phase l reads pipe l's stage for the self-loop
        # diag matmuls, so the two layers' stages must coexist
        self.stage = mm_sb.tile(
            [P, NBLK * P], mybir.dt.bfloat16,
            tag=f"mst_{bounce_d.name}", name=f"stage_{bounce_d.name}",
        )
        self.done_b = 0
        self.piece = 0

    def advance(self, bend):
        nc = self.nc
        dt = mybir.dt
        act = mybir.ActivationFunctionType
        for b in range(self.done_b, bend):
            nb = min(P, NPC - b * P)
            pm = self.mm_ps.tile([P, P], dt.float32, tag="pm")
            nc.tensor.matmul(
                pm[:nb, :],
                self.featT[:, b * P : b * P + nb],
                self.w_t[:],
                start=True,
                stop=True,
            )
            nc.scalar.activation(
                self.stage[:nb, b * P : (b + 1) * P], pm[:nb, :], act.Copy
            )
        self.done_b = bend

        while self.piece < CH and (
            (self.piece + 1) * MMB <= bend or bend == NBLK
        ):
            p = self.piece
            blo = p * MMB
            bhi = min(NBLK, blo + MMB)
            full = min(bhi * P, (NPC // P) * P)
            nc.sync.dma_start(
                self.bounce_d[blo * P : full, :].rearrange(
                    "(b p) f -> p b f", p=P
                ),
                self.stage[:, blo * P : full].rearrange("p (b f) -> p b f", f=P),
            )
            if bhi * P > full:          # tail remainder rows (12416..12500)
                rem = NPC - full
                nc.sync.dma_start(
                    self.bounce_d[full:, :], self.stage[:rem, full:]
                )
            nc.gpsimd.collective_compute(
                "AllGather",
                mybir.AluOpType.bypass,
                replica_groups=[list(range(NCORES))],
                ins=[self.bounce_d[p * QROWS : (p + 1) * QROWS, :]],
                outs=[self.tables_d[p][:]],
            )
            self.piece += 1


def _edge_phase(
    nc, tc, table_d, sched, idx_t, dst_t, nrm_t, iota_t, stag, spool, post,
    stage, dinv2_t, ident_t, after_group=None,
):
    dt = mybir.dt
    alu = mybir.AluOpType
    pieces = sched["pieces"]
    last_m_of_b = sched["last_m_of_b"]
    GTM = sched["GTM"]

    with tc.tile_pool(name="agg_ps", bufs=1, space="PSUM") as aps:
        for g in range(NGRP):
            blo, bhi = g * GRP, min(NBLK, (g + 1) * GRP)
            ng = bhi - blo
            pa = {}
            for b in range(blo, bhi):
                pa_b = aps.tile(
                    [P, P], dt.float32, tag=f"pa{b - blo}", name=f"pa_{g}_{b}"
                )
                pa[b] = pa_b
            # self-loops: pa[b] starts as stage[b]^T * dinv2 (diag select)
            sdg = spool.tile(
                [P, GRP, P], dt.bfloat16, tag="sdg", bufs=2, name="sdg"
            )
            nc.vector.tensor_tensor(
                sdg[:, :ng, :],
                ident_t[:].unsqueeze(1).broadcast_to([P, ng, P]),
                dinv2_t[:, blo:bhi].unsqueeze(2).broadcast_to([P, ng, P]),
                op=alu.mult,
            )
            for b in range(blo, bhi):
                nb = min(P, NPC - b * P)
                nc.tensor.matmul(
                    pa[b][:],
                    stage[:nb, b * P : (b + 1) * P],
                    sdg[:nb, b - blo, :],
                    start=True,
                    stop=(last_m_of_b[b] < 0),
                )
            for ch in range(CH):
                for pt0, pnt, m0, mlist in pieces[(g, ch)]:
                    if not mlist:
                        continue
                    stg = stag.tile([P, GT, P], dt.bfloat16, tag="stag")
                    nc.gpsimd.dma_gather(
                        stg[:, :pnt, :],
                        table_d[ch][:],
                        idx_t[:, pt0 * 8 : (pt0 + pnt) * 8],
                        pnt * P,
                        pnt * P,
                        P,
                        single_packet=False,
                        queue_num=ch,
                    )
                    # batched select build over this piece's matmul columns:
                    #   sel[e, m', j] = (iota[e, j] == dst[e, m0+m']) * nrm
                    mn = len(mlist)
                    sel = spool.tile([P, GTM, P], dt.bfloat16, tag="sel")
                    nc.vector.tensor_tensor(
                        sel[:, :mn, :],
                        iota_t[:].unsqueeze(1).broadcast_to([P, mn, P]),
                        dst_t[:, m0 : m0 + mn]
                        .unsqueeze(2)
                        .broadcast_to([P, mn, P]),
                        op=alu.is_equal,
                    )
                    nc.vector.tensor_tensor(
                        sel[:, :mn, :],
                        sel[:, :mn, :],
                        nrm_t[:, m0 : m0 + mn]
                        .unsqueeze(2)
                        .broadcast_to([P, mn, P]),
                        op=alu.mult,
                    )
                    for mrel, slot, b in mlist:
                        nc.tensor.matmul(
                            pa[b][:],
                            stg[:, slot, :],
                            sel[:, mrel, :],
                            start=False,
                            stop=(last_m_of_b[b] == m0 + mrel),
                        )
            for b in range(blo, bhi):
                nb = min(P, NPC - b * P)
                post(b, nb, pa[b])
            if after_group is not None:
                after_group(bhi)


# ---------------------------------------------------------------- entry


def build(z, edge_index, W_ih, W_hh, b_ih, b_hh, W1, b1, W2, b2, W3, b3):
    """Host prep + trace + compile. Returns (nc, in_maps)."""
    z = np.asarray(z, dtype=np.float32)
    W_ih = np.asarray(W_ih, dtype=np.float32)
    b = np.asarray(b_ih, dtype=np.float32) + np.asarray(b_hh, dtype=np.float32)

    arrs, sched = _prep_edges(edge_index)
    nc = _build_nc(sched)

    gi = {"i": 0, "g": 2, "o": 3}  # torch gate order i,f,g,o (f unused: c0=0)
    common = {
        "iota": np.ascontiguousarray(
            np.tile(np.arange(P, dtype=np.float32), (P, 1))
        ).astype(bf16),
        "ident": np.eye(P, dtype=np.float32).astype(bf16),
        "w1": np.asarray(W1, np.float32).astype(bf16),
        "w2": np.asarray(W2, np.float32).astype(bf16),
        "w3t": np.ascontiguousarray(np.asarray(W3, np.float32).T).astype(bf16),
        "b1": np.asarray(b1, np.float32).reshape(P, 1).copy(),
        "b2": np.asarray(b2, np.float32).reshape(P, 1).copy(),
        "b3": np.asarray(b3, np.float32).reshape(P, 1).copy(),
    }
    for g, k in gi.items():
        common[f"wih_{g}"] = np.ascontiguousarray(
            W_ih[k * P : (k + 1) * P, :].T
        ).astype(bf16)
        common[f"bg_{g}"] = b[k * P : (k + 1) * P].reshape(P, 1).copy()

    in_maps = []
    for c in range(NCORES):
        idx16, dstt, nrmt, d2t = arrs[c]
        m = dict(common)
        m["zT"] = np.ascontiguousarray(z[c * NPC : (c + 1) * NPC].T).astype(bf16)
        m["idx16"] = idx16
        m["dstv"] = dstt
        m["nrmv"] = nrmt
        m["dinv2"] = d2t
        in_maps.append(m)
    return nc, in_maps


def assemble(results):
    out = np.empty((N, P), np.float32)
    for c in range(NCORES):
        out[c * NPC : (c + 1) * NPC] = results[c]["outT"].T
    return out


def kernel(z, edge_index, W_ih, W_hh, b_ih, b_hh, W1, b1, W2, b2, W3, b3):
    nc, in_maps = build(z, edge_index, W_ih, W_hh, b_ih, b_hh, W1, b1, W2, b2, W3, b3)
    res = run_bass_kernel_spmd(nc, in_maps, core_ids=list(range(NCORES)))
    return assemble(res.results)


# revision 45
# speedup vs baseline: 17.8657x; 1.0131x over previous
"""Trainium2 Bass kernel for nn_Decoder (GNN message passing):
LSTM(1 step) -> GCNConv -> ReLU -> GCNConv -> Linear -> ReLU on a
100K-node / 1.6M-edge graph, SPMD across 8 NeuronCores.

Strategy (dst-node sharding):
- Core c owns nodes [c*12500, (c+1)*12500) and all edges into them.
- Per-node compute (LSTM, x@W transforms) runs feature-major [128, nodes]
  so all matmuls need zero transposes and biases are per-partition.
- The GCN propagate gathers transformed rows from a bf16 node-major table
  in DRAM (built via sub-AllGathers of the 8 shards, pipelined against the
  per-block transform matmuls) with gpsimd.dma_gather spread over 4 SWDGE
  queues (each queue runs on its own Q7 core pair), then scatter-adds via
  PE matmul with an on-chip selection matrix built in batched broadcast
  tensor_tensor ops (iota==dst -> * norm), accumulated in PSUM.
- Edge tiles are ordered group-major (groups of 8 dst blocks, chunk-major
  inside) so chunk-q gathers only wait on sub-AllGather q, and 8 PSUM
  accumulators carry a group across the 4 chunk sweeps.
"""

from contextlib import ExitStack

import numpy as np
import ml_dtypes

import concourse.bacc as bacc
import concourse.mybir as mybir
import concourse.tile as tile
from concourse.bass_utils import run_bass_kernel_spmd

P = 128
N = 100000
NCORES = 8
NPC = N // NCORES            # 12500 nodes per core
NBLK = (NPC + P - 1) // P    # 98 dst blocks per core (last has 84)
CH = 4                       # src chunks (int16 gather index limit)
QROWS = NPC // CH            # 3125: per-rank quarter contributed to a chunk
CHROWS = QROWS * NCORES      # 25000 rows per chunk table
GT = 24                      # max tiles (of 128 edges) per dma_gather
GRP = 6                      # dst blocks per PSUM accumulation group (6 PSUM
                             # banks for accumulators + 2 for the transform)
NGRP = (NBLK + GRP - 1) // GRP
MMB = 25                     # transform/bounce blocks per sub-AllGather piece
LSTM_CHUNK = 500             # nodes per LSTM/matmul column chunk

bf16 = ml_dtypes.bfloat16
f32 = np.float32


# ---------------------------------------------------------------- host prep


def _prep_edges(edge_index):
    """Sort/pad each core's incident edges into a cross-core-uniform tile
    schedule. Edges are packed densely per (dst-block group, src chunk)
    SEGMENT (tiles may cross dst-block boundaries inside a segment); the
    device runs one matmul per (tile, touched block) with per-matmul select
    columns precomputed here. Returns per-core arrays + the schedule."""
    src = np.asarray(edge_index[0], dtype=np.int64)
    dst = np.asarray(edge_index[1], dtype=np.int64)

    # self-loops enter deg/norm but are applied on-chip from the transform
    # stage (diag matmul), not gathered
    deg = np.bincount(dst, minlength=N).astype(np.float64) + 1.0
    dinv = 1.0 / np.sqrt(deg)
    norm = (dinv[src] * dinv[dst]).astype(np.float32)
    dinv2 = (dinv * dinv).astype(np.float32)

    NSEG = NGRP * CH

    core_of = dst // NPC
    per_core = []
    seg_cnt = np.zeros((NCORES, NSEG), np.int64)
    for c in range(NCORES):
        m = core_of == c
        s = src[m]
        d = dst[m] - c * NPC
        w = norm[m]
        ch = (s % NPC) // QROWS
        o = np.lexsort((d, ch))
        s, d, w, ch = s[o], d[o], w[o], ch[o]
        b = d // P
        sid = (b // GRP) * CH + ch
        seg_cnt[c] = np.bincount(sid, minlength=NSEG)
        per_core.append((s, d, w, ch, b, sid))

    # tiles per segment: dense packing, padded to the max across cores
    seg_tiles = (seg_cnt.max(axis=0) + P - 1) // P      # [NSEG]
    seg_base = np.concatenate([[0], np.cumsum(seg_tiles)[:-1]])
    TT = int(seg_tiles.sum())
    NIDX = TT * P

    # per-core slot position of each edge + (seg, tile-in-seg, block) triples
    core_pos = []
    touched = [set() for _ in range(NSEG)]              # (tloc, b) per segment
    for c in range(NCORES):
        s, d, w, ch, b, sid = per_core[c]
        # edges are sorted (ch, d) -> segments appear in ch-major order
        skey = ch * NGRP + (b // GRP)
        cnt_k = np.bincount(skey, minlength=NSEG)
        kstart = np.concatenate([[0], np.cumsum(cnt_k)[:-1]])
        within = np.arange(len(s)) - kstart[skey]
        pos = seg_base[sid] * P + within
        tloc = within // P
        core_pos.append((pos, tloc))
        for ss in range(NSEG):
            msk = sid == ss
            for tb in set(zip(tloc[msk].tolist(), b[msk].tolist())):
                touched[ss].add(tb)

    # matmul schedule: emission order (g, ch, tile, block); pieces of <= GT
    # tiles per gather call with their matmul lists
    mindex = {}                                        # (sid, tloc, b) -> m
    pieces = {}                                        # (g, ch) -> [piece...]
    last_m_of_b = np.full(NBLK, -1, np.int64)
    MT = 0
    GTM = 0
    for g in range(NGRP):
        for ch in range(CH):
            sid = g * CH + ch
            nt = int(seg_tiles[sid])
            t0 = int(seg_base[sid])
            tb_sorted = sorted(touched[sid])
            plist = []
            for k in range(0, nt, GT):
                pn = min(GT, nt - k)
                mlist = []
                m0 = MT
                for tloc, b in tb_sorted:
                    if k <= tloc < k + pn:
                        mindex[(sid, tloc, b)] = MT
                        mlist.append((MT - m0, tloc - k, b))
                        last_m_of_b[b] = MT
                        MT += 1
                plist.append((t0 + k, pn, m0, mlist))
                GTM = max(GTM, len(mlist))
            pieces[(g, ch)] = plist

    arrs = []
    for c in range(NCORES):
        s, d, w, ch, b, sid = per_core[c]
        pos, tloc = core_pos[c]

        idxs = np.zeros(NIDX, np.int16)                 # pad -> row 0 (valid)
        # chunk q table = concat over ranks of each rank's q-th quarter
        idxs[pos] = ((s // NPC) * QROWS + (s % QROWS)).astype(np.int16)

        marr = np.fromiter(
            (mindex[(int(ss), int(tt), int(bb))] for ss, tt, bb in zip(sid, tloc, b)),
            dtype=np.int64,
            count=len(s),
        )
        row = pos % P
        dstv = np.full(MT * P, -1.0, np.float32)        # default: no dst match
        dstv[marr * P + row] = (d - b * P).astype(np.float32)
        # norm is applied to the gathered rows (per slot), so the select
        # matrices are pure one-hots
        nrmv = np.zeros(NIDX, np.float32)               # pad slots scale to 0
        nrmv[pos] = w

        idx16 = np.tile(np.ascontiguousarray(idxs.reshape(-1, 16).T), (8, 1))
        dstt = np.ascontiguousarray(dstv.reshape(MT, P).T).astype(bf16)
        nrmt = np.ascontiguousarray(nrmv.reshape(TT, P).T).astype(bf16)
        d2 = np.zeros((NBLK, P), np.float32)          # [block, node-in-block]
        d2.reshape(-1)[:NPC] = dinv2[c * NPC : (c + 1) * NPC]
        d2t = np.ascontiguousarray(d2.T).astype(bf16)  # [P, NBLK]
        arrs.append((idx16, dstt, nrmt, d2t))

    sched = dict(
        TT=TT, NIDX=NIDX, MT=MT, GTM=GTM,
        pieces=pieces, last_m_of_b=last_m_of_b,
    )
    return arrs, sched


# ---------------------------------------------------------------- device


def _build_nc(sched):
    NIDX = sched["NIDX"]
    MT = sched["MT"]
    TT = sched["TT"]
    dt = mybir.dt
    alu = mybir.AluOpType
    act = mybir.ActivationFunctionType

    nc = bacc.Bacc(
        "TRN2",
        target_bir_lowering=False,
        debug=False,
        num_devices=NCORES,
        num_swdge_queues=4,
    )

    # ---- I/O
    zT_d = nc.dram_tensor("zT", [P, NPC], dt.bfloat16, kind="ExternalInput")
    idx_d = nc.dram_tensor("idx16", [P, NIDX // 16], dt.int16, kind="ExternalInput")
    dst_d = nc.dram_tensor("dstv", [P, MT], dt.bfloat16, kind="ExternalInput")
    nrm_d = nc.dram_tensor("nrmv", [P, TT], dt.bfloat16, kind="ExternalInput")
    iota_d = nc.dram_tensor("iota", [P, P], dt.bfloat16, kind="ExternalInput")
    ident_d = nc.dram_tensor("ident", [P, P], dt.bfloat16, kind="ExternalInput")
    dinv2_d = nc.dram_tensor("dinv2", [P, NBLK], dt.bfloat16, kind="ExternalInput")
    wih_d = {
        g: nc.dram_tensor(f"wih_{g}", [P, P], dt.bfloat16, kind="ExternalInput")
        for g in "igo"
    }
    bg_d = {
        g: nc.dram_tensor(f"bg_{g}", [P, 1], dt.float32, kind="ExternalInput")
        for g in "igo"
    }
    w1_d = nc.dram_tensor("w1", [P, P], dt.bfloat16, kind="ExternalInput")
    w2_d = nc.dram_tensor("w2", [P, P], dt.bfloat16, kind="ExternalInput")
    w3t_d = nc.dram_tensor("w3t", [P, P], dt.bfloat16, kind="ExternalInput")
    b1_d = nc.dram_tensor("b1", [P, 1], dt.float32, kind="ExternalInput")
    b2_d = nc.dram_tensor("b2", [P, 1], dt.float32, kind="ExternalInput")
    b3_d = nc.dram_tensor("b3", [P, 1], dt.float32, kind="ExternalInput")
    out_d = nc.dram_tensor("outT", [P, NPC], dt.float32, kind="ExternalOutput")

    bounce = [nc.dram_tensor(f"bounce{l}", [NPC, P], dt.bfloat16) for l in range(2)]
    table = [
        [
            nc.dram_tensor(
                f"table{l}_{q}", [CHROWS, P], dt.bfloat16, addr_space="Shared"
            )
            for q in range(CH)
        ]
        for l in range(2)
    ]

    with tile.TileContext(nc) as tc, ExitStack() as ctx:
        konst = ctx.enter_context(tc.tile_pool(name="konst", bufs=1))
        big = ctx.enter_context(tc.tile_pool(name="big", bufs=1))

        def load_const(handle, shape, dtype):
            t = konst.tile(shape, dtype, tag=handle.name)
            nc.sync.dma_start(t[:], handle[:])
            return t

        iota_t = load_const(iota_d, [P, P], dt.bfloat16)
        ident_t = load_const(ident_d, [P, P], dt.bfloat16)
        dinv2_t = load_const(dinv2_d, [P, NBLK], dt.bfloat16)
        wih_t = {g: load_const(wih_d[g], [P, P], dt.bfloat16) for g in "igo"}
        bg_t = {g: load_const(bg_d[g], [P, 1], dt.float32) for g in "igo"}
        w1_t = load_const(w1_d, [P, P], dt.bfloat16)
        w2_t = load_const(w2_d, [P, P], dt.bfloat16)
        w3t_t = load_const(w3t_d, [P, P], dt.bfloat16)
        b1_t = load_const(b1_d, [P, 1], dt.float32)
        b2_t = load_const(b2_d, [P, 1], dt.float32)
        b3_t = load_const(b3_d, [P, 1], dt.float32)
        idx_t = load_const(idx_d, [P, NIDX // 16], dt.int16)
        dst_t = load_const(dst_d, [P, MT], dt.bfloat16)
        nrm_t = load_const(nrm_d, [P, TT], dt.bfloat16)

        xT_t = big.tile([P, NPC], dt.bfloat16, tag="xT")  # x1T then x2T

        mm_ps = ctx.enter_context(tc.tile_pool(name="m_ps", bufs=2, space="PSUM"))
        mm_sb = ctx.enter_context(tc.tile_pool(name="m_sb", bufs=1))

        # ---------------- phase 1: LSTM -> hT (feature-major, bf16)
        with tc.tile_pool(name="h_pool", bufs=1) as hpool:
            hT_t = hpool.tile([P, NPC], dt.bfloat16, tag="hT")
            with (
                tc.tile_pool(name="lstm_sb", bufs=1) as lsb,
                tc.tile_pool(name="lstm_ps", bufs=6, space="PSUM") as lps,
                tc.tile_pool(name="lstm_tr", bufs=8) as ltr,
            ):
                zT_t = lsb.tile([P, NPC], dt.bfloat16, tag="zT")
                nc.sync.dma_start(zT_t[:], zT_d[:])

                pipe1 = _MMPipe(
                    nc, tc, mm_ps, mm_sb, hT_t, w1_t, bounce[0], table[0]
                )
                nchunk = (NPC + LSTM_CHUNK - 1) // LSTM_CHUNK
                for k in range(nchunk):
                    c0 = k * LSTM_CHUNK
                    c1 = min(NPC, c0 + LSTM_CHUNK)
                    w = c1 - c0
                    gate = {}
                    for g in "igo":
                        ps = lps.tile([P, LSTM_CHUNK], dt.float32, tag="ps")
                        nc.tensor.matmul(
                            ps[:, :w], wih_t[g][:], zT_t[:, c0:c1], start=True, stop=True
                        )
                        fn = act.Tanh if g == "g" else act.Sigmoid
                        sg = ltr.tile([P, LSTM_CHUNK], dt.bfloat16, tag="sg" + g)
                        nc.scalar.activation(sg[:, :w], ps[:, :w], fn, bias=bg_t[g][:])
                        gate[g] = sg
                    ct = ltr.tile([P, LSTM_CHUNK], dt.bfloat16, tag="ct")
                    nc.vector.tensor_tensor(
                        ct[:, :w], gate["i"][:, :w], gate["g"][:, :w], op=alu.mult
                    )
                    th = ltr.tile([P, LSTM_CHUNK], dt.bfloat16, tag="th")
                    nc.scalar.activation(th[:, :w], ct[:, :w], act.Tanh)
                    nc.vector.tensor_tensor(
                        hT_t[:, c0:c1], gate["o"][:, :w], th[:, :w], op=alu.mult
                    )
                    # phase 2 interleaved: transform blocks fully covered by
                    # the LSTM so sub-AllGathers start during the LSTM sweep
                    pipe1.advance(c1 // P)

            pipe1.advance(NBLK)

        with (
            tc.tile_pool(name="stag", bufs=7) as stag,
            tc.tile_pool(name="spool", bufs=4) as spool,
        ):
            # ------------- phase 3: edge layer 1 -> x1T = relu(agg + b1),
            # interleaved with phase 4 (m2 = x1 @ W2 -> bounce1 + AGs)
            def post1(b, nb, pa):
                nc.scalar.activation(
                    xT_t[:, b * P : b * P + nb], pa[:, :nb], act.Relu, bias=b1_t[:]
                )

            pipe2 = _MMPipe(nc, tc, mm_ps, mm_sb, xT_t, w2_t, bounce[1], table[1])
            _edge_phase(
                nc, tc, table[0], sched, idx_t, dst_t, nrm_t, iota_t,
                stag, spool, post1, pipe1.stage, dinv2_t, ident_t,
                after_group=pipe2.advance,
            )
            pipe2.advance(NBLK)

            # ------------- phase 5: edge layer 2 -> x2T = agg + b2 (no relu)
            def post2(b, nb, pa):
                nc.vector.tensor_scalar(
                    xT_t[:, b * P : b * P + nb], pa[:, :nb], b2_t[:], None, op0=alu.add
                )

            _edge_phase(
                nc, tc, table[1], sched, idx_t, dst_t, nrm_t, iota_t,
                stag, spool, post2, pipe2.stage, dinv2_t, ident_t,
            )

        # ---------------- phase 6: outT = relu(W3T.T @ x2T + b3)
        with (
            tc.tile_pool(name="out_ps", bufs=3, space="PSUM") as ops,
            tc.tile_pool(name="out_sb", bufs=3) as osb,
        ):
            nchunk = (NPC + LSTM_CHUNK - 1) // LSTM_CHUNK
            for k in range(nchunk):
                c0 = k * LSTM_CHUNK
                c1 = min(NPC, c0 + LSTM_CHUNK)
                w = c1 - c0
                ps = ops.tile([P, LSTM_CHUNK], dt.float32, tag="ps")
                nc.tensor.matmul(
                    ps[:, :w], w3t_t[:], xT_t[:, c0:c1], start=True, stop=True
                )
                ot = osb.tile([P, LSTM_CHUNK], dt.float32, tag="ot")
                nc.scalar.activation(ot[:, :w], ps[:, :w], act.Relu, bias=b3_t[:])
                nc.sync.dma_start(out_d[:, c0:c1], ot[:, :w])

    nc.compile()
    return nc


class _MMPipe:
    """Per-block transform (featT block @ W -> node-major bf16 stage),
    with bounce-DMA + sub-AllGather emitted per MMB-block piece so the
    collectives overlap trailing compute."""

    def __init__(self, nc, tc, mm_ps, mm_sb, featT, w_t, bounce_d, tables_d):
        self.nc = nc
        self.mm_ps = mm_ps
        self.mm_sb = mm_sb
        self.featT = featT
        self.w_t = w_t
        self.bounce_d = bounce_d
        self.tables_d = tables_d
        # per-pipe stage: edge phase l reads pipe l's stage for the self-loop
        # diag matmuls, so the two layers' stages must coexist
        self.stage = mm_sb.tile(
            [P, NBLK * P], mybir.dt.bfloat16,
            tag=f"mst_{bounce_d.name}", name=f"stage_{bounce_d.name}",
        )
        self.done_b = 0
        self.piece = 0

    def advance(self, bend):
        nc = self.nc
        dt = mybir.dt
        act = mybir.ActivationFunctionType
        for b in range(self.done_b, bend):
            nb = min(P, NPC - b * P)
            pm = self.mm_ps.tile([P, P], dt.float32, tag="pm")
            nc.tensor.matmul(
                pm[:nb, :],
                self.featT[:, b * P : b * P + nb],
                self.w_t[:],
                start=True,
                stop=True,
            )
            nc.scalar.activation(
                self.stage[:nb, b * P : (b + 1) * P], pm[:nb, :], act.Copy
            )
        self.done_b = bend

        while self.piece < CH and (
            (self.piece + 1) * MMB <= bend or bend == NBLK
        ):
            p = self.piece
            blo = p * MMB
            bhi = min(NBLK, blo + MMB)
            full = min(bhi * P, (NPC // P) * P)
            nc.sync.dma_start(
                self.bounce_d[blo * P : full, :].rearrange(
                    "(b p) f -> p b f", p=P
                ),
                self.stage[:, blo * P : full].rearrange("p (b f) -> p b f", f=P),
            )
            if bhi * P > full:          # tail remainder rows (12416..12500)
                rem = NPC - full
                nc.sync.dma_start(
                    self.bounce_d[full:, :], self.stage[:rem, full:]
                )
            nc.gpsimd.collective_compute(
                "AllGather",
                mybir.AluOpType.bypass,
                replica_groups=[list(range(NCORES))],
                ins=[self.bounce_d[p * QROWS : (p + 1) * QROWS, :]],
                outs=[self.tables_d[p][:]],
            )
            self.piece += 1


def _edge_phase(
    nc, tc, table_d, sched, idx_t, dst_t, nrm_t, iota_t, stag, spool, post,
    stage, dinv2_t, ident_t, after_group=None,
):
    dt = mybir.dt
    alu = mybir.AluOpType
    pieces = sched["pieces"]
    last_m_of_b = sched["last_m_of_b"]
    GTM = sched["GTM"]

    with tc.tile_pool(name="agg_ps", bufs=1, space="PSUM") as aps:
        for g in range(NGRP):
            blo, bhi = g * GRP, min(NBLK, (g + 1) * GRP)
            ng = bhi - blo
            pa = {}
            for b in range(blo, bhi):
                pa_b = aps.tile(
                    [P, P], dt.float32, tag=f"pa{b - blo}", name=f"pa_{g}_{b}"
                )
                pa[b] = pa_b
            # self-loops: pa[b] starts as stage[b]^T * dinv2 (diag select)
            sdg = spool.tile(
                [P, GRP, P], dt.bfloat16, tag="sdg", bufs=1, name="sdg"
            )
            nc.vector.tensor_tensor(
                sdg[:, :ng, :],
                ident_t[:].unsqueeze(1).broadcast_to([P, ng, P]),
                dinv2_t[:, blo:bhi].unsqueeze(2).broadcast_to([P, ng, P]),
                op=alu.mult,
            )
            for b in range(blo, bhi):
                nb = min(P, NPC - b * P)
                nc.tensor.matmul(
                    pa[b][:],
                    stage[:nb, b * P : (b + 1) * P],
                    sdg[:nb, b - blo, :],
                    start=True,
                    stop=(last_m_of_b[b] < 0),
                )
            for ch in range(CH):
                for pt0, pnt, m0, mlist in pieces[(g, ch)]:
                    if not mlist:
                        continue
                    stg = stag.tile([P, GT, P], dt.bfloat16, tag="stag")
                    nc.gpsimd.dma_gather(
                        stg[:, :pnt, :],
                        table_d[ch][:],
                        idx_t[:, pt0 * 8 : (pt0 + pnt) * 8],
                        pnt * P,
                        pnt * P,
                        P,
                        single_packet=False,
                        queue_num=ch,
                    )
                    # scale gathered rows by their edge norm (pad rows -> 0)
                    nc.vector.tensor_tensor(
                        stg[:, :pnt, :],
                        stg[:, :pnt, :],
                        nrm_t[:, pt0 : pt0 + pnt]
                        .unsqueeze(2)
                        .broadcast_to([P, pnt, P]),
                        op=alu.mult,
                    )
                    # batched one-hot select build per matmul column:
                    #   sel[e, m', j] = (iota[e, j] == dst[e, m0+m'])
                    mn = len(mlist)
                    sel = spool.tile([P, GTM, P], dt.bfloat16, tag="sel")
                    nc.vector.tensor_tensor(
                        sel[:, :mn, :],
                        iota_t[:].unsqueeze(1).broadcast_to([P, mn, P]),
                        dst_t[:, m0 : m0 + mn]
                        .unsqueeze(2)
                        .broadcast_to([P, mn, P]),
                        op=alu.is_equal,
                    )
                    for mrel, slot, b in mlist:
                        nc.tensor.matmul(
                            pa[b][:],
                            stg[:, slot, :],
                            sel[:, mrel, :],
                            start=False,
                            stop=(last_m_of_b[b] == m0 + mrel),
                        )
            for b in range(blo, bhi):
                nb = min(P, NPC - b * P)
                post(b, nb, pa[b])
            if after_group is not None:
                after_group(bhi)


# ---------------------------------------------------------------- entry


def build(z, edge_index, W_ih, W_hh, b_ih, b_hh, W1, b1, W2, b2, W3, b3):
    """Host prep + trace + compile. Returns (nc, in_maps)."""
    z = np.asarray(z, dtype=np.float32)
    W_ih = np.asarray(W_ih, dtype=np.float32)
    b = np.asarray(b_ih, dtype=np.float32) + np.asarray(b_hh, dtype=np.float32)

    arrs, sched = _prep_edges(edge_index)
    nc = _build_nc(sched)

    gi = {"i": 0, "g": 2, "o": 3}  # torch gate order i,f,g,o (f unused: c0=0)
    common = {
        "iota": np.ascontiguousarray(
            np.tile(np.arange(P, dtype=np.float32), (P, 1))
        ).astype(bf16),
        "ident": np.eye(P, dtype=np.float32).astype(bf16),
        "w1": np.asarray(W1, np.float32).astype(bf16),
        "w2": np.asarray(W2, np.float32).astype(bf16),
        "w3t": np.ascontiguousarray(np.asarray(W3, np.float32).T).astype(bf16),
        "b1": np.asarray(b1, np.float32).reshape(P, 1).copy(),
        "b2": np.asarray(b2, np.float32).reshape(P, 1).copy(),
        "b3": np.asarray(b3, np.float32).reshape(P, 1).copy(),
    }
    for g, k in gi.items():
        common[f"wih_{g}"] = np.ascontiguousarray(
            W_ih[k * P : (k + 1) * P, :].T
        ).astype(bf16)
        common[f"bg_{g}"] = b[k * P : (k + 1) * P].reshape(P, 1).copy()

    in_maps = []
    for c in range(NCORES):
        idx16, dstt, nrmt, d2t = arrs[c]
        m = dict(common)
        m["zT"] = np.ascontiguousarray(z[c * NPC : (c + 1) * NPC].T).astype(bf16)
        m["idx16"] = idx16
        m["dstv"] = dstt
        m["nrmv"] = nrmt
        m["dinv2"] = d2t
        in_maps.append(m)
    return nc, in_maps


def assemble(results):
    out = np.empty((N, P), np.float32)
    for c in range(NCORES):
        out[c * NPC : (c + 1) * NPC] = results[c]["outT"].T
    return out


def kernel(z, edge_index, W_ih, W_hh, b_ih, b_hh, W1, b1, W2, b2, W3, b3):
    nc, in_maps = build(z, edge_index, W_ih, W_hh, b_ih, b_hh, W1, b1, W2, b2, W3, b3)
    res = run_bass_kernel_spmd(nc, in_maps, core_ids=list(range(NCORES)))
    return assemble(res.results)


# revision 47
# speedup vs baseline: 18.8549x; 1.0554x over previous
"""Trainium2 Bass kernel for nn_Decoder (GNN message passing):
LSTM(1 step) -> GCNConv -> ReLU -> GCNConv -> Linear -> ReLU on a
100K-node / 1.6M-edge graph, SPMD across 8 NeuronCores.

Strategy (dst-node sharding):
- Core c owns nodes [c*12500, (c+1)*12500) and all edges into them.
- Per-node compute (LSTM, x@W transforms) runs feature-major [128, nodes]
  so all matmuls need zero transposes and biases are per-partition.
- The GCN propagate gathers transformed rows from a bf16 node-major table
  in DRAM (built via sub-AllGathers of the 8 shards, pipelined against the
  per-block transform matmuls) with gpsimd.dma_gather spread over 4 SWDGE
  queues (each queue runs on its own Q7 core pair), then scatter-adds via
  PE matmul with an on-chip selection matrix built in batched broadcast
  tensor_tensor ops (iota==dst -> * norm), accumulated in PSUM.
- Edge tiles are ordered group-major (groups of 8 dst blocks, chunk-major
  inside) so chunk-q gathers only wait on sub-AllGather q, and 8 PSUM
  accumulators carry a group across the 4 chunk sweeps.
"""

from contextlib import ExitStack

import numpy as np
import ml_dtypes

import concourse.bacc as bacc
import concourse.mybir as mybir
import concourse.tile as tile
from concourse.bass_utils import run_bass_kernel_spmd

P = 128
N = 100000
NCORES = 8
NPC = N // NCORES            # 12500 nodes per core
NBLK = (NPC + P - 1) // P    # 98 dst blocks per core (last has 84)
CH = 4                       # src chunks (int16 gather index limit)
QROWS = NPC // CH            # 3125: per-rank quarter contributed to a chunk
CHROWS = QROWS * NCORES      # 25000 rows per chunk table
GT = 28                      # max tiles (of 128 edges) per dma_gather
                             # (segments average ~24 tiles -> mostly 1 call)
GRP = 6                      # dst blocks per PSUM accumulation group (6 PSUM
                             # banks for accumulators + 2 for the transform)
NGRP = (NBLK + GRP - 1) // GRP
MMB = 25                     # transform/bounce blocks per sub-AllGather piece
LSTM_CHUNK = 500             # nodes per LSTM/matmul column chunk

bf16 = ml_dtypes.bfloat16
f32 = np.float32


# ---------------------------------------------------------------- host prep


def _prep_edges(edge_index):
    """Sort/pad each core's incident edges into a cross-core-uniform tile
    schedule. Edges are packed densely per (dst-block group, src chunk)
    SEGMENT (tiles may cross dst-block boundaries inside a segment); the
    device runs one matmul per (tile, touched block) with per-matmul select
    columns precomputed here. Returns per-core arrays + the schedule."""
    src = np.asarray(edge_index[0], dtype=np.int64)
    dst = np.asarray(edge_index[1], dtype=np.int64)

    # self-loops enter deg/norm but are applied on-chip from the transform
    # stage (diag matmul), not gathered
    deg = np.bincount(dst, minlength=N).astype(np.float64) + 1.0
    dinv = 1.0 / np.sqrt(deg)
    norm = (dinv[src] * dinv[dst]).astype(np.float32)
    dinv2 = (dinv * dinv).astype(np.float32)

    NSEG = NGRP * CH

    core_of = dst // NPC
    per_core = []
    seg_cnt = np.zeros((NCORES, NSEG), np.int64)
    for c in range(NCORES):
        m = core_of == c
        s = src[m]
        d = dst[m] - c * NPC
        w = norm[m]
        ch = (s % NPC) // QROWS
        o = np.lexsort((d, ch))
        s, d, w, ch = s[o], d[o], w[o], ch[o]
        b = d // P
        sid = (b // GRP) * CH + ch
        seg_cnt[c] = np.bincount(sid, minlength=NSEG)
        per_core.append((s, d, w, ch, b, sid))

    # tiles per segment: dense packing, padded to the max across cores
    seg_tiles = (seg_cnt.max(axis=0) + P - 1) // P      # [NSEG]
    seg_base = np.concatenate([[0], np.cumsum(seg_tiles)[:-1]])
    TT = int(seg_tiles.sum())
    NIDX = TT * P

    # per-core slot position of each edge + (seg, tile-in-seg, block) triples
    core_pos = []
    touched = [set() for _ in range(NSEG)]              # (tloc, b) per segment
    for c in range(NCORES):
        s, d, w, ch, b, sid = per_core[c]
        # edges are sorted (ch, d) -> segments appear in ch-major order
        skey = ch * NGRP + (b // GRP)
        cnt_k = np.bincount(skey, minlength=NSEG)
        kstart = np.concatenate([[0], np.cumsum(cnt_k)[:-1]])
        within = np.arange(len(s)) - kstart[skey]
        pos = seg_base[sid] * P + within
        tloc = within // P
        core_pos.append((pos, tloc))
        for ss in range(NSEG):
            msk = sid == ss
            for tb in set(zip(tloc[msk].tolist(), b[msk].tolist())):
                touched[ss].add(tb)

    # matmul schedule: emission order (g, ch, tile, block); pieces of <= GT
    # tiles per gather call with their matmul lists
    mindex = {}                                        # (sid, tloc, b) -> m
    pieces = {}                                        # (g, ch) -> [piece...]
    last_m_of_b = np.full(NBLK, -1, np.int64)
    MT = 0
    GTM = 0
    for g in range(NGRP):
        for ch in range(CH):
            sid = g * CH + ch
            nt = int(seg_tiles[sid])
            t0 = int(seg_base[sid])
            tb_sorted = sorted(touched[sid])
            plist = []
            for k in range(0, nt, GT):
                pn = min(GT, nt - k)
                mlist = []
                m0 = MT
                for tloc, b in tb_sorted:
                    if k <= tloc < k + pn:
                        mindex[(sid, tloc, b)] = MT
                        mlist.append((MT - m0, tloc - k, b))
                        last_m_of_b[b] = MT
                        MT += 1
                plist.append((t0 + k, pn, m0, mlist))
                GTM = max(GTM, len(mlist))
            pieces[(g, ch)] = plist

    arrs = []
    for c in range(NCORES):
        s, d, w, ch, b, sid = per_core[c]
        pos, tloc = core_pos[c]

        idxs = np.zeros(NIDX, np.int16)                 # pad -> row 0 (valid)
        # chunk q table = concat over ranks of each rank's q-th quarter
        idxs[pos] = ((s // NPC) * QROWS + (s % QROWS)).astype(np.int16)

        marr = np.fromiter(
            (mindex[(int(ss), int(tt), int(bb))] for ss, tt, bb in zip(sid, tloc, b)),
            dtype=np.int64,
            count=len(s),
        )
        row = pos % P
        dstv = np.full(MT * P, -1.0, np.float32)        # default: no dst match
        dstv[marr * P + row] = (d - b * P).astype(np.float32)
        # norm is applied to the gathered rows (per slot), so the select
        # matrices are pure one-hots
        nrmv = np.zeros(NIDX, np.float32)               # pad slots scale to 0
        nrmv[pos] = w

        idx16 = np.tile(np.ascontiguousarray(idxs.reshape(-1, 16).T), (8, 1))
        dstt = np.ascontiguousarray(dstv.reshape(MT, P).T).astype(bf16)
        nrmt = np.ascontiguousarray(nrmv.reshape(TT, P).T).astype(bf16)
        d2 = np.zeros((NBLK, P), np.float32)          # [block, node-in-block]
        d2.reshape(-1)[:NPC] = dinv2[c * NPC : (c + 1) * NPC]
        d2t = np.ascontiguousarray(d2.T).astype(bf16)  # [P, NBLK]
        arrs.append((idx16, dstt, nrmt, d2t))

    sched = dict(
        TT=TT, NIDX=NIDX, MT=MT, GTM=GTM,
        pieces=pieces, last_m_of_b=last_m_of_b,
    )
    return arrs, sched


# ---------------------------------------------------------------- device


def _build_nc(sched):
    NIDX = sched["NIDX"]
    MT = sched["MT"]
    TT = sched["TT"]
    dt = mybir.dt
    alu = mybir.AluOpType
    act = mybir.ActivationFunctionType

    nc = bacc.Bacc(
        "TRN2",
        target_bir_lowering=False,
        debug=False,
        num_devices=NCORES,
        num_swdge_queues=4,
    )

    # ---- I/O
    zT_d = nc.dram_tensor("zT", [P, NPC], dt.bfloat16, kind="ExternalInput")
    idx_d = nc.dram_tensor("idx16", [P, NIDX // 16], dt.int16, kind="ExternalInput")
    dst_d = nc.dram_tensor("dstv", [P, MT], dt.bfloat16, kind="ExternalInput")
    nrm_d = nc.dram_tensor("nrmv", [P, TT], dt.bfloat16, kind="ExternalInput")
    iota_d = nc.dram_tensor("iota", [P, P], dt.bfloat16, kind="ExternalInput")
    ident_d = nc.dram_tensor("ident", [P, P], dt.bfloat16, kind="ExternalInput")
    dinv2_d = nc.dram_tensor("dinv2", [P, NBLK], dt.bfloat16, kind="ExternalInput")
    wih_d = {
        g: nc.dram_tensor(f"wih_{g}", [P, P], dt.bfloat16, kind="ExternalInput")
        for g in "igo"
    }
    bg_d = {
        g: nc.dram_tensor(f"bg_{g}", [P, 1], dt.float32, kind="ExternalInput")
        for g in "igo"
    }
    w1_d = nc.dram_tensor("w1", [P, P], dt.bfloat16, kind="ExternalInput")
    w2_d = nc.dram_tensor("w2", [P, P], dt.bfloat16, kind="ExternalInput")
    w3t_d = nc.dram_tensor("w3t", [P, P], dt.bfloat16, kind="ExternalInput")
    b1_d = nc.dram_tensor("b1", [P, 1], dt.float32, kind="ExternalInput")
    b2_d = nc.dram_tensor("b2", [P, 1], dt.float32, kind="ExternalInput")
    b3_d = nc.dram_tensor("b3", [P, 1], dt.float32, kind="ExternalInput")
    out_d = nc.dram_tensor("outT", [P, NPC], dt.float32, kind="ExternalOutput")

    bounce = [nc.dram_tensor(f"bounce{l}", [NPC, P], dt.bfloat16) for l in range(2)]
    table = [
        [
            nc.dram_tensor(
                f"table{l}_{q}", [CHROWS, P], dt.bfloat16, addr_space="Shared"
            )
            for q in range(CH)
        ]
        for l in range(2)
    ]

    with tile.TileContext(nc) as tc, ExitStack() as ctx:
        konst = ctx.enter_context(tc.tile_pool(name="konst", bufs=1))
        big = ctx.enter_context(tc.tile_pool(name="big", bufs=1))

        def load_const(handle, shape, dtype):
            t = konst.tile(shape, dtype, tag=handle.name)
            nc.sync.dma_start(t[:], handle[:])
            return t

        iota_t = load_const(iota_d, [P, P], dt.bfloat16)
        ident_t = load_const(ident_d, [P, P], dt.bfloat16)
        dinv2_t = load_const(dinv2_d, [P, NBLK], dt.bfloat16)
        wih_t = {g: load_const(wih_d[g], [P, P], dt.bfloat16) for g in "igo"}
        bg_t = {g: load_const(bg_d[g], [P, 1], dt.float32) for g in "igo"}
        w1_t = load_const(w1_d, [P, P], dt.bfloat16)
        w2_t = load_const(w2_d, [P, P], dt.bfloat16)
        w3t_t = load_const(w3t_d, [P, P], dt.bfloat16)
        b1_t = load_const(b1_d, [P, 1], dt.float32)
        b2_t = load_const(b2_d, [P, 1], dt.float32)
        b3_t = load_const(b3_d, [P, 1], dt.float32)
        idx_t = load_const(idx_d, [P, NIDX // 16], dt.int16)
        dst_t = load_const(dst_d, [P, MT], dt.bfloat16)
        nrm_t = load_const(nrm_d, [P, TT], dt.bfloat16)

        xT_t = big.tile([P, NPC], dt.bfloat16, tag="xT")  # x1T then x2T

        mm_ps = ctx.enter_context(tc.tile_pool(name="m_ps", bufs=2, space="PSUM"))
        mm_sb = ctx.enter_context(tc.tile_pool(name="m_sb", bufs=1))

        # ---------------- phase 1: LSTM -> hT (feature-major, bf16)
        with tc.tile_pool(name="h_pool", bufs=1) as hpool:
            hT_t = hpool.tile([P, NPC], dt.bfloat16, tag="hT")
            with (
                tc.tile_pool(name="lstm_sb", bufs=1) as lsb,
                tc.tile_pool(name="lstm_ps", bufs=6, space="PSUM") as lps,
                tc.tile_pool(name="lstm_tr", bufs=8) as ltr,
            ):
                zT_t = lsb.tile([P, NPC], dt.bfloat16, tag="zT")
                nc.sync.dma_start(zT_t[:], zT_d[:])

                pipe1 = _MMPipe(
                    nc, tc, mm_ps, mm_sb, hT_t, w1_t, bounce[0], table[0]
                )
                nchunk = (NPC + LSTM_CHUNK - 1) // LSTM_CHUNK
                for k in range(nchunk):
                    c0 = k * LSTM_CHUNK
                    c1 = min(NPC, c0 + LSTM_CHUNK)
                    w = c1 - c0
                    gate = {}
                    for g in "igo":
                        ps = lps.tile([P, LSTM_CHUNK], dt.float32, tag="ps")
                        nc.tensor.matmul(
                            ps[:, :w], wih_t[g][:], zT_t[:, c0:c1], start=True, stop=True
                        )
                        fn = act.Tanh if g == "g" else act.Sigmoid
                        sg = ltr.tile([P, LSTM_CHUNK], dt.bfloat16, tag="sg" + g)
                        nc.scalar.activation(sg[:, :w], ps[:, :w], fn, bias=bg_t[g][:])
                        gate[g] = sg
                    ct = ltr.tile([P, LSTM_CHUNK], dt.bfloat16, tag="ct")
                    nc.vector.tensor_tensor(
                        ct[:, :w], gate["i"][:, :w], gate["g"][:, :w], op=alu.mult
                    )
                    th = ltr.tile([P, LSTM_CHUNK], dt.bfloat16, tag="th")
                    nc.scalar.activation(th[:, :w], ct[:, :w], act.Tanh)
                    nc.vector.tensor_tensor(
                        hT_t[:, c0:c1], gate["o"][:, :w], th[:, :w], op=alu.mult
                    )
                    # phase 2 interleaved: transform blocks fully covered by
                    # the LSTM so sub-AllGathers start during the LSTM sweep
                    pipe1.advance(c1 // P)

            pipe1.advance(NBLK)

        with (
            tc.tile_pool(name="stag", bufs=6) as stag,
            tc.tile_pool(name="spool", bufs=3) as spool,
        ):
            # ------------- phase 3: edge layer 1 -> x1T = relu(agg + b1),
            # interleaved with phase 4 (m2 = x1 @ W2 -> bounce1 + AGs)
            def post1(b, nb, pa):
                nc.scalar.activation(
                    xT_t[:, b * P : b * P + nb], pa[:, :nb], act.Relu, bias=b1_t[:]
                )

            pipe2 = _MMPipe(nc, tc, mm_ps, mm_sb, xT_t, w2_t, bounce[1], table[1])
            _edge_phase(
                nc, tc, table[0], sched, idx_t, dst_t, nrm_t, iota_t,
                stag, spool, post1, pipe1.stage, dinv2_t, ident_t,
                after_group=pipe2.advance,
            )
            pipe2.advance(NBLK)

            # ------------- phase 5: edge layer 2 -> x2T = agg + b2 (no relu)
            def post2(b, nb, pa):
                nc.vector.tensor_scalar(
                    xT_t[:, b * P : b * P + nb], pa[:, :nb], b2_t[:], None, op0=alu.add
                )

            _edge_phase(
                nc, tc, table[1], sched, idx_t, dst_t, nrm_t, iota_t,
                stag, spool, post2, pipe2.stage, dinv2_t, ident_t,
            )

        # ---------------- phase 6: outT = relu(W3T.T @ x2T + b3)
        with (
            tc.tile_pool(name="out_ps", bufs=3, space="PSUM") as ops,
            tc.tile_pool(name="out_sb", bufs=3) as osb,
        ):
            nchunk = (NPC + LSTM_CHUNK - 1) // LSTM_CHUNK
            for k in range(nchunk):
                c0 = k * LSTM_CHUNK
                c1 = min(NPC, c0 + LSTM_CHUNK)
                w = c1 - c0
                ps = ops.tile([P, LSTM_CHUNK], dt.float32, tag="ps")
                nc.tensor.matmul(
                    ps[:, :w], w3t_t[:], xT_t[:, c0:c1], start=True, stop=True
                )
                ot = osb.tile([P, LSTM_CHUNK], dt.float32, tag="ot")
                nc.scalar.activation(ot[:, :w], ps[:, :w], act.Relu, bias=b3_t[:])
                nc.sync.dma_start(out_d[:, c0:c1], ot[:, :w])

    nc.compile()
    return nc


class _MMPipe:
    """Per-block transform (featT block @ W -> node-major bf16 stage),
    with bounce-DMA + sub-AllGather emitted per MMB-block piece so the
    collectives overlap trailing compute."""

    def __init__(self, nc, tc, mm_ps, mm_sb, featT, w_t, bounce_d, tables_d):
        self.nc = nc
        self.mm_ps = mm_ps
        self.mm_sb = mm_sb
        self.featT = featT
        self.w_t = w_t
        self.bounce_d = bounce_d
        self.tables_d = tables_d
        # per-pipe stage: edge phase l reads pipe l's stage for the self-loop
        # diag matmuls, so the two layers' stages must coexist
        self.stage = mm_sb.tile(
            [P, NBLK * P], mybir.dt.bfloat16,
            tag=f"mst_{bounce_d.name}", name=f"stage_{bounce_d.name}",
        )
        self.done_b = 0
        self.piece = 0

    def advance(self, bend):
        nc = self.nc
        dt = mybir.dt
        act = mybir.ActivationFunctionType
        for b in range(self.done_b, bend):
            nb = min(P, NPC - b * P)
            pm = self.mm_ps.tile([P, P], dt.float32, tag="pm")
            nc.tensor.matmul(
                pm[:nb, :],
                self.featT[:, b * P : b * P + nb],
                self.w_t[:],
                start=True,
                stop=True,
            )
            nc.scalar.activation(
                self.stage[:nb, b * P : (b + 1) * P], pm[:nb, :], act.Copy
            )
        self.done_b = bend

        while self.piece < CH and (
            (self.piece + 1) * MMB <= bend or bend == NBLK
        ):
            p = self.piece
            blo = p * MMB
            bhi = min(NBLK, blo + MMB)
            full = min(bhi * P, (NPC // P) * P)
            nc.sync.dma_start(
                self.bounce_d[blo * P : full, :].rearrange(
                    "(b p) f -> p b f", p=P
                ),
                self.stage[:, blo * P : full].rearrange("p (b f) -> p b f", f=P),
            )
            if bhi * P > full:          # tail remainder rows (12416..12500)
                rem = NPC - full
                nc.sync.dma_start(
                    self.bounce_d[full:, :], self.stage[:rem, full:]
                )
            nc.gpsimd.collective_compute(
                "AllGather",
                mybir.AluOpType.bypass,
                replica_groups=[list(range(NCORES))],
                ins=[self.bounce_d[p * QROWS : (p + 1) * QROWS, :]],
                outs=[self.tables_d[p][:]],
            )
            self.piece += 1


def _edge_phase(
    nc, tc, table_d, sched, idx_t, dst_t, nrm_t, iota_t, stag, spool, post,
    stage, dinv2_t, ident_t, after_group=None,
):
    dt = mybir.dt
    alu = mybir.AluOpType
    pieces = sched["pieces"]
    last_m_of_b = sched["last_m_of_b"]
    GTM = sched["GTM"]

    with tc.tile_pool(name="agg_ps", bufs=1, space="PSUM") as aps:
        for g in range(NGRP):
            blo, bhi = g * GRP, min(NBLK, (g + 1) * GRP)
            ng = bhi - blo
            pa = {}
            for b in range(blo, bhi):
                pa_b = aps.tile(
                    [P, P], dt.float32, tag=f"pa{b - blo}", name=f"pa_{g}_{b}"
                )
                pa[b] = pa_b
            # self-loops: pa[b] starts as stage[b]^T * dinv2 (diag select)
            sdg = spool.tile(
                [P, GRP, P], dt.bfloat16, tag="sdg", bufs=1, name="sdg"
            )
            nc.vector.tensor_tensor(
                sdg[:, :ng, :],
                ident_t[:].unsqueeze(1).broadcast_to([P, ng, P]),
                dinv2_t[:, blo:bhi].unsqueeze(2).broadcast_to([P, ng, P]),
                op=alu.mult,
            )
            for b in range(blo, bhi):
                nb = min(P, NPC - b * P)
                nc.tensor.matmul(
                    pa[b][:],
                    stage[:nb, b * P : (b + 1) * P],
                    sdg[:nb, b - blo, :],
                    start=True,
                    stop=(last_m_of_b[b] < 0),
                )
            for ch in range(CH):
                for pt0, pnt, m0, mlist in pieces[(g, ch)]:
                    if not mlist:
                        continue
                    stg = stag.tile([P, GT, P], dt.bfloat16, tag="stag")
                    nc.gpsimd.dma_gather(
                        stg[:, :pnt, :],
                        table_d[ch][:],
                        idx_t[:, pt0 * 8 : (pt0 + pnt) * 8],
                        pnt * P,
                        pnt * P,
                        P,
                        single_packet=False,
                        queue_num=ch,
                    )
                    # scale gathered rows by their edge norm (pad rows -> 0)
                    nc.vector.tensor_tensor(
                        stg[:, :pnt, :],
                        stg[:, :pnt, :],
                        nrm_t[:, pt0 : pt0 + pnt]
                        .unsqueeze(2)
                        .broadcast_to([P, pnt, P]),
                        op=alu.mult,
                    )
                    # batched one-hot select build per matmul column:
                    #   sel[e, m', j] = (iota[e, j] == dst[e, m0+m'])
                    mn = len(mlist)
                    sel = spool.tile([P, GTM, P], dt.bfloat16, tag="sel")
                    nc.vector.tensor_tensor(
                        sel[:, :mn, :],
                        iota_t[:].unsqueeze(1).broadcast_to([P, mn, P]),
                        dst_t[:, m0 : m0 + mn]
                        .unsqueeze(2)
                        .broadcast_to([P, mn, P]),
                        op=alu.is_equal,
                    )
                    for mrel, slot, b in mlist:
                        nc.tensor.matmul(
                            pa[b][:],
                            stg[:, slot, :],
                            sel[:, mrel, :],
                            start=False,
                            stop=(last_m_of_b[b] == m0 + mrel),
                        )
            for b in range(blo, bhi):
                nb = min(P, NPC - b * P)
                post(b, nb, pa[b])
            if after_group is not None:
                after_group(bhi)


# ---------------------------------------------------------------- entry


def build(z, edge_index, W_ih, W_hh, b_ih, b_hh, W1, b1, W2, b2, W3, b3):
    """Host prep + trace + compile. Returns (nc, in_maps)."""
    z = np.asarray(z, dtype=np.float32)
    W_ih = np.asarray(W_ih, dtype=np.float32)
    b = np.asarray(b_ih, dtype=np.float32) + np.asarray(b_hh, dtype=np.float32)

    arrs, sched = _prep_edges(edge_index)
    nc = _build_nc(sched)

    gi = {"i": 0, "g": 2, "o": 3}  # torch gate order i,f,g,o (f unused: c0=0)
    common = {
        "iota": np.ascontiguousarray(
            np.tile(np.arange(P, dtype=np.float32), (P, 1))
        ).astype(bf16),
        "ident": np.eye(P, dtype=np.float32).astype(bf16),
        "w1": np.asarray(W1, np.float32).astype(bf16),
        "w2": np.asarray(W2, np.float32).astype(bf16),
        "w3t": np.ascontiguousarray(np.asarray(W3, np.float32).T).astype(bf16),
        "b1": np.asarray(b1, np.float32).reshape(P, 1).copy(),
        "b2": np.asarray(b2, np.float32).reshape(P, 1).copy(),
        "b3": np.asarray(b3, np.float32).reshape(P, 1).copy(),
    }
    for g, k in gi.items():
        common[f"wih_{g}"] = np.ascontiguousarray(
            W_ih[k * P : (k + 1) * P, :].T
        ).astype(bf16)
        common[f"bg_{g}"] = b[k * P : (k + 1) * P].reshape(P, 1).copy()

    in_maps = []
    for c in range(NCORES):
        idx16, dstt, nrmt, d2t = arrs[c]
        m = dict(common)
        m["zT"] = np.ascontiguousarray(z[c * NPC : (c + 1) * NPC].T).astype(bf16)
        m["idx16"] = idx16
        m["dstv"] = dstt
        m["nrmv"] = nrmt
        m["dinv2"] = d2t
        in_maps.append(m)
    return nc, in_maps


def assemble(results):
    out = np.empty((N, P), np.float32)
    for c in range(NCORES):
        out[c * NPC : (c + 1) * NPC] = results[c]["outT"].T
    return out


def kernel(z, edge_index, W_ih, W_hh, b_ih, b_hh, W1, b1, W2, b2, W3, b3):
    nc, in_maps = build(z, edge_index, W_ih, W_hh, b_ih, b_hh, W1, b1, W2, b2, W3, b3)
    res = run_bass_kernel_spmd(nc, in_maps, core_ids=list(range(NCORES)))
    return assemble(res.results)
